# revision 10
# baseline (speedup 1.0000x reference)
"""Trainium2 Bass kernel for nn_CgsNodeFeat (gaussian-mixture graph conv).

Full-input contract: kernel(**inputs) takes the complete arrays, shards
batch-wise across 8 NeuronCores (4 batches each), runs one SPMD Bass
program, and reassembles the full [32, 128, 1024] output.

Math per (batch b, node n):
  centre_j   = node_centre[b, idx[b,n,k]]                 (gather)
  rho,theta  = polar(node_centre[b,n] - centre_j)
  gauss[k,m] = exp(-.5 (rho-mu_r[m])^2 s_r[m]) * exp(-.5 wrap(theta-mu_t[m])^2 s_t[m])
  w[k,m]     = graph_weights[b,n,k] * gauss[k,m] / sum_m gauss[k,m]
  out[n]     = relu( concat_m( (sum_k w[k,m] F[b, idx[b,n,k], m-block]) ) )
  with F[b]  = node_feats[b] @ Wmat,  Wmat[c, m*128+d] = conv_w[m, c, d]

Device mapping (per core, 4 batches, everything on 128 partitions):
  - mask2[n,(j,k)] = (j == idx[n,k])            one DVE is_equal, bf16
  - maskT[j,n] per k via DMA transpose          (for the centre gather)
  - centre gather on PE: matmul(maskT_k, centre6) with centres split
    hi/mid/lo in bf16 so the fp32 values are reproduced exactly
  - rho/theta/gaussian/normalise: small DVE/ACT ops on [n,16]/[n,128]
  - Wdelta_k[n',(n,m)] = Ident(n,n') * w[n',k,m] (DVE 2x, bf16)
  - AT_stack[j,(n,m)] = sum_k mask2_k.T @ Wdelta_k   (PE, PSUM accum)
  - F[j,(m,d)] = X^T tiles @ Wmat tiles              (PE, bf16)
  - out[n, m*128+d] = matmul(AT[:, (:,m)], F[:, m-block]); relu on ACT
"""
import os
import sys

sys.path.insert(0, "/opt/trn_rl_repo")

import numpy as np
from contextlib import ExitStack

import concourse.bass as bass
import concourse.tile as tile
from concourse import bacc, mybir
from concourse.bass_utils import run_bass_kernel_spmd

F32 = mybir.dt.float32
U8 = mybir.dt.uint8
BF16 = mybir.dt.bfloat16
I32 = mybir.dt.int32

N_CORES = 8
B, N, K, C, M, OUT = 32, 128, 16, 1024, 8, 1024
BPC = int(os.environ.get("KERNEL_BPC", B // N_CORES))   # batches per core
D = OUT // M                # 128
TWO_PI = 2.0 * float(np.pi)
EPS = 1e-14
PI = float(np.pi)

_CACHE = {}


def _build_nc():
    nc = bacc.Bacc("TRN2", target_bir_lowering=False, debug=False, num_devices=N_CORES)

    # ---- external tensors ----
    x_in = nc.dram_tensor("x_in", [BPC, N, C], F32, kind="ExternalInput")
    idx_in = nc.dram_tensor("idx_in", [BPC, N, K], I32, kind="ExternalInput")
    gw_in = nc.dram_tensor("gw_in", [BPC, N, K], F32, kind="ExternalInput")
    c6_in = nc.dram_tensor("c6_in", [BPC, N, 8], BF16, kind="ExternalInput")   # xh xm xl yh ym yl pad pad
    ccol_in = nc.dram_tensor("ccol_in", [BPC, N, 2], F32, kind="ExternalInput")
    gp_in = nc.dram_tensor("gp_in", [N, 32], F32, kind="ExternalInput")        # mu_r | mu_t | p_r | p_t (replicated)
    iota2_in = nc.dram_tensor("iota2_in", [N, N * K], BF16, kind="ExternalInput")  # value j at (j,k)
    idm_in = nc.dram_tensor("idm_in", [N, N * M], BF16, kind="ExternalInput")  # delta(n==n') at (n,m)
    wmat_in = nc.dram_tensor("wmat_in", [C, OUT], BF16, kind="ExternalInput")  # [c, m*128+d]
    y_out = nc.dram_tensor("y_out", [BPC, N, OUT], F32, kind="ExternalOutput")

    with tile.TileContext(nc) as tc, ExitStack() as ctx:
        cpool = ctx.enter_context(tc.tile_pool(name="const", bufs=1))
        wpool = ctx.enter_context(tc.tile_pool(name="wmat", bufs=1))
        xpool = ctx.enter_context(tc.tile_pool(name="x", bufs=2))
        epool = ctx.enter_context(tc.tile_pool(name="edge", bufs=2))
        mpool = ctx.enter_context(tc.tile_pool(name="mask", bufs=2))
        tpool = ctx.enter_context(tc.tile_pool(name="mt", bufs=2))
        opool = ctx.enter_context(tc.tile_pool(name="out", bufs=2))
        pp_f = ctx.enter_context(tc.tile_pool(name="ps_f", bufs=1, space="PSUM"))
        pp_a = ctx.enter_context(tc.tile_pool(name="ps_a", bufs=1, space="PSUM"))
        pp_o = ctx.enter_context(tc.tile_pool(name="ps_o", bufs=1, space="PSUM"))
        pp_g = ctx.enter_context(tc.tile_pool(name="ps_g", bufs=1, space="PSUM"))

        # ---- constants ----
        iota2 = cpool.tile([N, N * K], BF16)
        nc.sync.dma_start(iota2[:], iota2_in[:])
        idm = cpool.tile([N, N * M], BF16)
        nc.sync.dma_start(idm[:], idm_in[:])
        gp = cpool.tile([N, 32], F32)
        nc.sync.dma_start(gp[:], gp_in[:])
        wmat = wpool.tile([128, C // 128, OUT], BF16)  # [p, ct, out]
        nc.sync.dma_start(wmat[:], wmat_in[:].rearrange("(a p) o -> p a o", p=128))

        # gaussian scale tables: s = -0.5 / (eps + p^2), on [N, 8] slices
        mu_r = gp[:, 0:8]
        mu_t = gp[:, 8:16]
        sc = cpool.tile([N, 16], F32)   # [s_r | s_t]
        nc.vector.tensor_tensor(sc[:], gp[:, 16:32], gp[:, 16:32], mybir.AluOpType.mult)
        nc.vector.tensor_scalar_add(sc[:], sc[:], EPS)
        nc.vector.reciprocal(sc[:], sc[:])
        nc.vector.tensor_scalar_mul(sc[:], sc[:], -0.5)
        s_r = sc[:, 0:8]
        s_t = sc[:, 8:16]

        for b in range(BPC):
            # ---- loads ----
            x_f = xpool.tile([N, C], F32, tag="x_f")
            nc.sync.dma_start(x_f[:], x_in[b])
            idx_t = epool.tile([N, K], I32, tag="idx")
            nc.sync.dma_start(idx_t[:], idx_in[b])
            gw_t = epool.tile([N, K], F32, tag="gw")
            nc.sync.dma_start(gw_t[:], gw_in[b])
            c6 = epool.tile([N, 8], BF16, tag="c6")
            nc.sync.dma_start(c6[:], c6_in[b])
            ccol = epool.tile([N, 2], F32, tag="ccol")
            nc.sync.dma_start(ccol[:], ccol_in[b])

            # ---- masks ----
            idxf = epool.tile([N, K], BF16, tag="idxf")
            nc.vector.tensor_copy(idxf[:], idx_t[:])
            mask1 = mpool.tile([N, K, N], BF16, tag="mask1")  # [n, k, j], j contiguous
            nc.vector.tensor_tensor(
                mask1[:],
                iota2[:].rearrange("p (k j) -> p k j", k=K),
                idxf[:].unsqueeze(2).broadcast_to([N, K, N]),
                mybir.AluOpType.is_equal,
            )

            # maskT per k via DMA transpose (contiguous source slice)
            maskT = tpool.tile([N, K, N], BF16, tag="maskT")  # [j, k, n]
            for k in range(K):
                nc.sync.dma_start_transpose(maskT[:, k, :], mask1[:, k, :])

            # ---- centre gather on PE: cg[n, k*8 + c] = sum_j maskT[k][j,n]^T . c6[j, c] ----
            cg_ps = pp_g.tile([N, K, 8], F32, tag="cg")
            for k in range(K):
                nc.tensor.matmul(cg_ps[:, k, :], maskT[:, k, :], c6[:], start=True, stop=True)
            # sum hi/mid/lo triples -> cenj [n, (k,2)]
            cenj = epool.tile([N, K, 2], F32, tag="cenj")
            nc.vector.tensor_reduce(
                cenj[:].rearrange("p a b -> p (a b)"),
                cg_ps[:, :, 0:6].rearrange("p k (c s) -> p k c s", s=3),
                mybir.AxisListType.X,
                mybir.AluOpType.add,
            )

            # ---- polar coords ----
            cx = epool.tile([N, K], F32, tag="cx")
            nc.vector.tensor_tensor(cx[:], ccol[:, 0:1].broadcast_to([N, K]), cenj[:, :, 0], mybir.AluOpType.subtract)
            cy = epool.tile([N, K], F32, tag="cy")
            nc.vector.tensor_tensor(cy[:], ccol[:, 1:2].broadcast_to([N, K]), cenj[:, :, 1], mybir.AluOpType.subtract)

            rho = epool.tile([N, K], F32, tag="rho")
            nc.vector.tensor_tensor(rho[:], cx[:], cx[:], mybir.AluOpType.mult)
            t0 = epool.tile([N, K], F32, tag="t0")
            nc.vector.tensor_tensor(t0[:], cy[:], cy[:], mybir.AluOpType.mult)
            nc.vector.tensor_tensor(rho[:], rho[:], t0[:], mybir.AluOpType.add)
            nc.scalar.activation(rho[:], rho[:], mybir.ActivationFunctionType.Sqrt)

            # atan2(cx, cy)
            ax = epool.tile([N, K], F32, tag="ax")
            nc.vector.tensor_scalar_mul(ax[:], cx[:], -1.0)
            nc.vector.tensor_tensor(ax[:], ax[:], cx[:], mybir.AluOpType.max)
            ay = epool.tile([N, K], F32, tag="ay")
            nc.vector.tensor_scalar_mul(ay[:], cy[:], -1.0)
            nc.vector.tensor_tensor(ay[:], ay[:], cy[:], mybir.AluOpType.max)
            mn = epool.tile([N, K], F32, tag="mn")
            nc.vector.tensor_tensor(mn[:], ax[:], ay[:], mybir.AluOpType.min)
            mx = epool.tile([N, K], F32, tag="mx")
            nc.vector.tensor_tensor(mx[:], ax[:], ay[:], mybir.AluOpType.max)
            nc.vector.tensor_scalar_add(mx[:], mx[:], 1e-37)
            rat = epool.tile([N, K], F32, tag="rat")
            nc.vector.reciprocal(rat[:], mx[:])
            nc.vector.tensor_tensor(rat[:], rat[:], mn[:], mybir.AluOpType.mult)
            th = epool.tile([N, K], F32, tag="th")
            nc.scalar.activation(th[:], rat[:], mybir.ActivationFunctionType.Arctan)
            cond = epool.tile([N, K], U8, tag="cond")
            alt = epool.tile([N, K], F32, tag="alt")
            sgn = epool.tile([N, K], F32, tag="sgn")
            nc.vector.tensor_tensor(cond[:], ax[:], ay[:], mybir.AluOpType.is_gt)
            nc.vector.tensor_scalar(alt[:], th[:], -1.0, PI / 2, mybir.AluOpType.mult, mybir.AluOpType.add)
            nc.vector.select(th[:], cond[:], alt[:], th[:])
            nc.vector.tensor_scalar(cond[:], cy[:], 0.0, None, mybir.AluOpType.is_lt)
            nc.vector.tensor_scalar(alt[:], th[:], -1.0, PI, mybir.AluOpType.mult, mybir.AluOpType.add)
            nc.vector.select(th[:], cond[:], alt[:], th[:])
            nc.vector.tensor_scalar(sgn[:], cx[:], 0.0, 2.0, mybir.AluOpType.is_ge, mybir.AluOpType.mult)
            nc.vector.tensor_scalar_add(sgn[:], sgn[:], -1.0)
            nc.vector.tensor_tensor(th[:], th[:], sgn[:], mybir.AluOpType.mult)

            # ---- gaussian weights: e[n, k, m] ----
            e1 = epool.tile([N, K, M], F32, tag="e1")
            nc.vector.tensor_tensor(
                e1[:], rho[:].unsqueeze(2).broadcast_to([N, K, M]),
                mu_r.unsqueeze(1).broadcast_to([N, K, M]), mybir.AluOpType.subtract)
            nc.vector.tensor_tensor(e1[:], e1[:], e1[:], mybir.AluOpType.mult)
            nc.vector.tensor_tensor(
                e1[:], e1[:], s_r.unsqueeze(1).broadcast_to([N, K, M]), mybir.AluOpType.mult)

            e2 = epool.tile([N, K, M], F32, tag="e2")
            nc.vector.tensor_tensor(
                e2[:], th[:].unsqueeze(2).broadcast_to([N, K, M]),
                mu_t.unsqueeze(1).broadcast_to([N, K, M]), mybir.AluOpType.subtract)
            nege2 = epool.tile([N, K, M], F32, tag="nege2")
            nc.vector.tensor_scalar_mul(nege2[:], e2[:], -1.0)
            nc.vector.tensor_tensor(e2[:], e2[:], nege2[:], mybir.AluOpType.max)
            a2t = epool.tile([N, K, M], F32, tag="a2t")
            nc.vector.tensor_scalar(a2t[:], e2[:], -1.0, TWO_PI, mybir.AluOpType.mult, mybir.AluOpType.add)
            nc.vector.tensor_tensor(e2[:], e2[:], a2t[:], mybir.AluOpType.min)
            nc.vector.tensor_tensor(e2[:], e2[:], e2[:], mybir.AluOpType.mult)
            nc.vector.tensor_tensor(
                e2[:], e2[:], s_t.unsqueeze(1).broadcast_to([N, K, M]), mybir.AluOpType.mult)

            nc.vector.tensor_tensor(e1[:], e1[:], e2[:], mybir.AluOpType.add)
            wg = epool.tile([N, K, M], F32, tag="wg")
            nc.scalar.activation(
                wg[:].rearrange("p a b -> p (a b)"),
                e1[:].rearrange("p a b -> p (a b)"),
                mybir.ActivationFunctionType.Exp)

            # normalise over m and fold in graph weights
            ssum = epool.tile([N, K], F32, tag="ssum")
            nc.vector.tensor_reduce(ssum[:], wg[:], mybir.AxisListType.X, mybir.AluOpType.add)
            gws = epool.tile([N, K], F32, tag="gws")
            nc.vector.reciprocal(gws[:], ssum[:])
            nc.vector.tensor_tensor(gws[:], gws[:], gw_t[:], mybir.AluOpType.mult)
            wt = epool.tile([N, K, M], BF16, tag="wt")  # w-tilde, bf16 [n, k, m]
            nc.vector.tensor_tensor(
                wt[:], wg[:], gws[:].unsqueeze(2).broadcast_to([N, K, M]), mybir.AluOpType.mult)

            # ---- Wdelta + PE collapse: AT_stack[j, (n,m)] ----
            at_ps = pp_a.tile([N, N * M], F32, tag="at")
            for k in range(K):
                wd = mpool.tile([N, N, M], BF16, tag="wd")
                nc.vector.tensor_tensor(
                    wd[:],
                    idm[:].rearrange("p (a b) -> p a b", b=M),
                    wt[:, k, :].unsqueeze(1).broadcast_to([N, N, M]),
                    mybir.AluOpType.mult,
                )
                wd_f = wd[:].rearrange("p a b -> p (a b)")
                for h in range(2):
                    nc.tensor.matmul(
                        at_ps[:, h * 512:(h + 1) * 512],
                        mask1[:, k, :],
                        wd_f[:, h * 512:(h + 1) * 512],
                        start=(k == 0), stop=(k == K - 1),
                    )
            at_sb = tpool.tile([N, N * M], BF16, tag="at_sb")
            nc.scalar.copy(at_sb[:], at_ps[:])

            # ---- F = X @ Wmat  (PE, bf16) ----
            x_bf = xpool.tile([N, C], BF16, tag="x_bf")
            nc.scalar.copy(x_bf[:], x_f[:])
            xT = xpool.tile([128, C // 128, N], BF16, tag="xT")
            for ct in range(C // 128):
                nc.sync.dma_start_transpose(xT[:, ct, :], x_bf[:, ct * 128:(ct + 1) * 128])
            f_ps = pp_f.tile([N, OUT], F32, tag="f")
            for ct in range(C // 128):
                for h in range(2):
                    nc.tensor.matmul(
                        f_ps[:, h * 512:(h + 1) * 512],
                        xT[:, ct, :],
                        wmat[:, ct, h * 512:(h + 1) * 512],
                        start=(ct == 0), stop=(ct == C // 128 - 1),
                    )
            f_bf = xpool.tile([N, OUT], BF16, tag="f_bf")
            nc.scalar.copy(f_bf[:], f_ps[:])

            # ---- final: out[n, m*128+d] ----
            o_ps = pp_o.tile([N, OUT], F32, tag="o")
            at_v = at_sb[:].rearrange("p (a b) -> p a b", b=M)   # [j, n, m]
            for m in range(M):
                nc.tensor.matmul(
                    o_ps[:, m * D:(m + 1) * D],
                    at_v[:, :, m],
                    f_bf[:, m * D:(m + 1) * D],
                    start=True, stop=True,
                )
            y_sb = opool.tile([N, OUT], F32, tag="y")
            nc.scalar.activation(y_sb[:], o_ps[:], mybir.ActivationFunctionType.Relu)
            nc.sync.dma_start(y_out[b], y_sb[:])

    nc.finalize()
    return nc


def _split3(v):
    """split fp32 array into three bf16 planes summing (almost) exactly to v"""
    import ml_dtypes
    hi = v.astype(ml_dtypes.bfloat16)
    r1 = v - hi.astype(np.float32)
    mid = r1.astype(ml_dtypes.bfloat16)
    lo = (r1 - mid.astype(np.float32)).astype(ml_dtypes.bfloat16)
    return hi, mid, lo


def _prep_shared(conv_w, mean_rho, mean_theta, precision_rho, precision_theta):
    import ml_dtypes
    wmat = np.ascontiguousarray(conv_w.transpose(1, 0, 2).reshape(C, OUT)).astype(ml_dtypes.bfloat16)
    gp = np.concatenate([mean_rho[0], mean_theta[0], precision_rho[0], precision_theta[0]]).astype(np.float32)
    gp = np.tile(gp[None, :], (N, 1))
    iota2 = np.tile(np.arange(N, dtype=np.float32), K)[None, :].repeat(N, 0).astype(ml_dtypes.bfloat16)
    idm = np.repeat(np.eye(N, dtype=np.float32), M, axis=1).astype(ml_dtypes.bfloat16)  # [n', n*M+m]
    return wmat, gp, iota2, idm


def kernel(node_feats, node_centre, neighbor_idx, graph_weights,
           mean_rho, mean_theta, precision_rho, precision_theta, conv_w):
    import ml_dtypes
    node_feats = np.asarray(node_feats, dtype=np.float32)
    node_centre = np.asarray(node_centre, dtype=np.float32)
    neighbor_idx = np.asarray(neighbor_idx, dtype=np.int32)
    graph_weights = np.asarray(graph_weights, dtype=np.float32)

    if "nc" not in _CACHE:
        _CACHE["nc"] = _build_nc()
    nc = _CACHE["nc"]

    wmat, gp, iota2, idm = _prep_shared(
        np.asarray(conv_w, dtype=np.float32),
        np.asarray(mean_rho, dtype=np.float32), np.asarray(mean_theta, dtype=np.float32),
        np.asarray(precision_rho, dtype=np.float32), np.asarray(precision_theta, dtype=np.float32))

    xh, xm, xl = _split3(node_centre[..., 0])
    yh, ym, yl = _split3(node_centre[..., 1])
    c6 = np.stack([xh, xm, xl, yh, ym, yl,
                   np.zeros_like(xh), np.zeros_like(xh)], axis=-1)  # [B, N, 8] bf16

    in_maps = []
    for core in range(N_CORES):
        s = slice(core * BPC, (core + 1) * BPC)
        in_maps.append({
            "x_in": node_feats[s],
            "idx_in": neighbor_idx[s],
            "gw_in": graph_weights[s],
            "c6_in": c6[s],
            "ccol_in": node_centre[s],
            "gp_in": gp,
            "iota2_in": iota2,
            "idm_in": idm,
            "wmat_in": wmat,
        })

    res = run_bass_kernel_spmd(nc, in_maps, list(range(N_CORES)),
                               trace=bool(int(os.environ.get("KERNEL_TRACE", "0"))))
    out = np.concatenate([res.results[i]["y_out"] for i in range(N_CORES)], axis=0)
    _CACHE["last_exec_time_ns"] = res.exec_time_ns
    return out


# revision 43
# speedup vs baseline: 90.6084x; 90.6084x over previous
"""Trainium2 Bass kernel for nn_CgsNodeFeat (gaussian-mixture graph conv).

kernel(**inputs) takes the full arrays, shards batch-wise across 8
NeuronCores (4 batches each), runs one SPMD Bass program via
run_bass_kernel_spmd, and reassembles the full [32, 128, 1024] output.

Math per (batch b, node n):
  centre_j   = node_centre[b, idx[b,n,k]]                 (gather)
  rho,theta  = polar(node_centre[b,n] - centre_j)
  gauss[k,m] = exp(-.5 (rho-mu_r[m])^2 s_r[m] - .5 wrap(theta-mu_t[m])^2 s_t[m])
  w[k,m]     = graph_weights[b,n,k] * gauss[k,m] / sum_m gauss[k,m]
  out[n]     = relu( concat_m( sum_k w[k,m] F[b, idx[b,n,k], m-block] ) )
  with F[b]  = node_feats[b] @ Wmat,  Wmat[c, m*128+d] = conv_w[m, c, d]

Device mapping (per core, all tiles on 128 partitions, bf16 matmuls):
  - maskT[j,(b,k,n)] = (j == idx[n,k]): one DMA partition-broadcast of the
    host-transposed index tensor + one 4x tensor_scalar is_equal
  - mask1[n,(k,j)]: is_equal against an on-device iota table
  - centre gather on PE: matmul(maskT_bk, centre6) with centres split
    hi/mid/lo in bf16 so fp32 coordinates are reproduced exactly
  - polar/atan2 (DVE poly, deg-7 in t^2) + gaussian + normalisation:
    batched across all 4 batches as [128, 64]/[128, 512] DVE ops; theta
    wrap via the add_range_wrap custom DVE op; single ACT table
    (ln/exp/copy/relu), sqrt(s) computed as exp(0.5 ln s)
  - Wdelta_k[n',(n,m)] = Ident_m(n,n') * w[n',k,m]  (DVE 2x, bf16)
  - AT_stack[j,(n,m)] = sum_k mask1_k.T @ Wdelta_k  (PE, PSUM accum)
  - F[j,(m,d)] = X^T tiles @ Wmat tiles; X^T via PE transpose
  - out[n, m*128+d] = matmul(AT[:,(:,m)], F[:, m-block]); relu on ACT

Cost-model makespan per core: ~88 us (from 151 us for the first working
version). DVE is the bottleneck engine (~61 us busy: Wdelta expansion 38,
masks ~8, gaussian/polar ~10).
"""
import os
import sys

sys.path.insert(0, "/opt/trn_rl_repo")

import numpy as np
from contextlib import ExitStack

import concourse.bass as bass
import concourse.tile as tile
from concourse import bacc, mybir
from concourse.bass_utils import run_bass_kernel_spmd

F32 = mybir.dt.float32
U8 = mybir.dt.uint8
BF16 = mybir.dt.bfloat16
I32 = mybir.dt.int32

N_CORES = 8
B, N, K, C, M, OUT = 32, 128, 16, 1024, 8, 1024
BPC = int(os.environ.get("KERNEL_BPC", B // N_CORES))   # batches per core
D = OUT // M                # 128
TWO_PI = 2.0 * float(np.pi)
EPS = 1e-14
PI = float(np.pi)

_CACHE = {}


def _build_nc():
    nc = bacc.Bacc("TRN2", target_bir_lowering=False, debug=False, num_devices=N_CORES)

    # ---- external tensors ----
    x_in = nc.dram_tensor("x_in", [BPC, N, C], F32, kind="ExternalInput")
    idx_in = nc.dram_tensor("idx_in", [BPC, N, K], I32, kind="ExternalInput")
    idxT_in = nc.dram_tensor("idxT_in", [BPC, K, N], BF16, kind="ExternalInput")
    gw_in = nc.dram_tensor("gw_in", [BPC, N, K], F32, kind="ExternalInput")
    c6_in = nc.dram_tensor("c6_in", [BPC, N, 8], BF16, kind="ExternalInput")   # xh xm xl yh ym yl pad pad
    ccol_in = nc.dram_tensor("ccol_in", [BPC, N, 2], F32, kind="ExternalInput")
    gp_in = nc.dram_tensor("gp_in", [N, 32], F32, kind="ExternalInput")        # mu_r | mu_t | p_r | p_t (replicated)
    pcol_in = nc.dram_tensor("pcol_in", [N, 1], F32, kind="ExternalInput")     # partition index column
    wmat_in = nc.dram_tensor("wmat_in", [C, OUT], BF16, kind="ExternalInput")  # [c, m*128+d]
    y_out = nc.dram_tensor("y_out", [BPC, N, OUT], F32, kind="ExternalOutput")

    with tile.TileContext(nc) as tc, ExitStack() as ctx:
        cpool = ctx.enter_context(tc.tile_pool(name="const", bufs=1))
        wpool = ctx.enter_context(tc.tile_pool(name="wmat", bufs=1))
        xpool = ctx.enter_context(tc.tile_pool(name="x", bufs=4))
        epool = ctx.enter_context(tc.tile_pool(name="edge", bufs=4))
        mpool = ctx.enter_context(tc.tile_pool(name="mask", bufs=4))
        tpool = ctx.enter_context(tc.tile_pool(name="mt", bufs=2))
        opool = ctx.enter_context(tc.tile_pool(name="out", bufs=2))
        pp_fo = ctx.enter_context(tc.tile_pool(name="ps_fo", bufs=2, space="PSUM"))
        pp_a = ctx.enter_context(tc.tile_pool(name="ps_a", bufs=2, space="PSUM"))

        # ---- constants (generated on device) ----
        I16 = mybir.dt.int16
        ii_t = cpool.tile([N, N * K], I16)
        nc.gpsimd.iota(ii_t[:], pattern=[[0, K], [1, N]], base=0, channel_multiplier=0)
        iota2 = cpool.tile([N, N * K], BF16)
        nc.vector.tensor_copy(iota2[:], ii_t[:])
        id_i = cpool.tile([N, N], I16)
        nc.gpsimd.iota(id_i[:], pattern=[[1, N]], base=0, channel_multiplier=-1)
        ident = cpool.tile([N, N], BF16)
        nc.vector.tensor_scalar(ident[:], id_i[:], 0.0, None, mybir.AluOpType.is_equal)
        di_t = cpool.tile([N, N * M], I16)
        nc.gpsimd.iota(di_t[:], pattern=[[1, N], [0, M]], base=0, channel_multiplier=-1)
        idm = cpool.tile([N, N * M], BF16)
        nc.vector.tensor_scalar(idm[:], di_t[:], 0.0, None, mybir.AluOpType.is_equal)
        gp = cpool.tile([N, 32], F32)
        nc.sync.dma_start(gp[:], gp_in[:])
        pcol = cpool.tile([N, 1], F32)
        nc.sync.dma_start(pcol[:], pcol_in[:])
        wmat = wpool.tile([128, C // 128, OUT], BF16)  # [p, ct, out]
        wmat_v = wmat_in[:].rearrange("(a p) o -> p a o", p=128)

        # gaussian scale tables: s = -0.5 / (eps + p^2), on [N, 8] slices
        mu_r = gp[:, 0:8]
        mu_t = gp[:, 8:16]
        sc = cpool.tile([N, 16], F32)   # [s_r | s_t]
        nc.vector.tensor_tensor(sc[:], gp[:, 16:32], gp[:, 16:32], mybir.AluOpType.mult)
        nc.vector.tensor_scalar_add(sc[:], sc[:], EPS)
        nc.vector.reciprocal(sc[:], sc[:])
        nc.vector.tensor_scalar_mul(sc[:], sc[:], -0.5)
        s_r = sc[:, 0:8]
        s_t = sc[:, 8:16]

        REPEAT = int(os.environ.get("KERNEL_REPEAT", "1"))
        AT_C = [9.999999228e-01, -3.333223260e-01, 1.997402841e-01, -1.404782037e-01,
                1.000220305e-01, -6.087445219e-02, 2.533168027e-02, -5.021058170e-03]
        BK = BPC * K      # batched edge width (64)
        BKM = BK * M      # 512

        def bview(t):
            # [N, BPC, K, ...] view helpers use raw slicing on 3/4-d tiles
            return t

        for rep in range(REPEAT):
            st = [dict() for _ in range(BPC)]

            # ---- phase 0: small loads first, then idxrep broadcast + masks ----
            gw_t = epool.tile([N, BPC, K], F32, tag="gw", name="gw", bufs=2)
            nc.sync.dma_start(gw_t[:], gw_in[:].rearrange("b n k -> n b k"))
            ccol = epool.tile([N, BPC, 2], F32, tag="ccol", name="ccol", bufs=2)
            for b in range(BPC):
                v = st[b]
                v["idx_t"] = epool.tile([N, K], I32, tag="idx", name="idx")
                nc.sync.dma_start(v["idx_t"][:], idx_in[b])
                v["c6"] = epool.tile([N, 8], BF16, tag="c6", name="c6")
                nc.sync.dma_start(v["c6"][:], c6_in[b])
                nc.sync.dma_start(ccol[:, b, :], ccol_in[b])

            for b in range(BPC):
                v = st[b]
                idx_t = v.pop("idx_t")
                idxf = epool.tile([N, K], BF16, tag="idxf", name="idxf")
                nc.vector.tensor_copy(idxf[:], idx_t[:])
                v["mask1"] = mpool.tile([N, K, N], BF16, tag="mask1", name="mask1")
                nc.vector.tensor_tensor(
                    v["mask1"][:],
                    iota2[:].rearrange("p (k j) -> p k j", k=K),
                    idxf[:].unsqueeze(2).broadcast_to([N, K, N]),
                    mybir.AluOpType.is_equal)

            idxrep = tpool.tile([N, BPC * K * N], BF16, tag="idxrep", name="idxrep", bufs=1)
            nc.scalar.dma_start(
                idxrep[:],
                idxT_in[:].rearrange("b k n -> (b k n)").unsqueeze(0).broadcast_to([N, BPC * K * N]))
            maskT = mpool.tile([N, BPC, K, N], BF16, tag="maskT", name="maskT", bufs=1)
            nc.vector.tensor_scalar(
                maskT[:].rearrange("p a b c -> p (a b c)"), idxrep[:], pcol[:], None,
                mybir.AluOpType.is_equal)

            for b in range(BPC):
                v = st[b]
                v["x_f"] = xpool.tile([N, C], F32, tag="x_f", name="x_f")
                nc.sync.dma_start(v["x_f"][:], x_in[b])

            # ---- phase 1: centre gather (PE) + batched polar/gaussian (DVE) ----
            cg_ps = pp_a.tile([N, BPC, K, 8], F32, tag="at", name="cg")
            for b in range(BPC):
                for k in range(K):
                    nc.tensor.matmul(cg_ps[:, b, k, :], maskT[:, b, k, :],
                                     st[b]["c6"][:], start=True, stop=True)
            cenj = epool.tile([N, BPC, K, 2], F32, tag="cenj", name="cenj", bufs=2)
            nc.vector.tensor_reduce(
                cenj[:].rearrange("p a b c -> p (a b c)"),
                cg_ps[:, :, :, 0:6].rearrange("p a k (c s) -> p a k c s", s=3),
                mybir.AxisListType.X, mybir.AluOpType.add)

            # polar: all batches at once on [N, BK]
            cx = epool.tile([N, BPC, K], F32, tag="cx", name="cx", bufs=2)
            nc.vector.tensor_tensor(
                cx[:], ccol[:, :, 0].unsqueeze(2).broadcast_to([N, BPC, K]),
                cenj[:, :, :, 0], mybir.AluOpType.subtract)
            cy = epool.tile([N, BPC, K], F32, tag="cy", name="cy", bufs=2)
            nc.vector.tensor_tensor(
                cy[:], ccol[:, :, 1].unsqueeze(2).broadcast_to([N, BPC, K]),
                cenj[:, :, :, 1], mybir.AluOpType.subtract)
            cxf = cx[:].rearrange("p a b -> p (a b)")
            cyf = cy[:].rearrange("p a b -> p (a b)")

            rho = epool.tile([N, BK], F32, tag="rho", name="rho", bufs=2)
            nc.vector.tensor_tensor(rho[:], cxf, cxf, mybir.AluOpType.mult)
            t0 = epool.tile([N, BK], F32, tag="t0", name="t0", bufs=2)
            nc.vector.tensor_tensor(t0[:], cyf, cyf, mybir.AluOpType.mult)
            nc.vector.tensor_tensor(rho[:], rho[:], t0[:], mybir.AluOpType.add)
            nc.vector.tensor_scalar_add(rho[:], rho[:], 1e-37)
            nc.scalar.activation(rho[:], rho[:], mybir.ActivationFunctionType.Ln)
            nc.scalar.activation(rho[:], rho[:], mybir.ActivationFunctionType.Exp, scale=0.5)

            ax = epool.tile([N, BK], F32, tag="ax", name="ax", bufs=2)
            nc.vector.tensor_scalar_mul(ax[:], cxf, -1.0)
            nc.vector.tensor_tensor(ax[:], ax[:], cxf, mybir.AluOpType.max)
            ay = epool.tile([N, BK], F32, tag="ay", name="ay", bufs=2)
            nc.vector.tensor_scalar_mul(ay[:], cyf, -1.0)
            nc.vector.tensor_tensor(ay[:], ay[:], cyf, mybir.AluOpType.max)
            mn = epool.tile([N, BK], F32, tag="mn", name="mn", bufs=2)
            nc.vector.tensor_tensor(mn[:], ax[:], ay[:], mybir.AluOpType.min)
            mx = epool.tile([N, BK], F32, tag="mx", name="mx", bufs=2)
            nc.vector.tensor_tensor(mx[:], ax[:], ay[:], mybir.AluOpType.max)
            nc.vector.tensor_scalar_add(mx[:], mx[:], 1e-37)
            rat = epool.tile([N, BK], F32, tag="rat", name="rat", bufs=2)
            nc.vector.reciprocal(rat[:], mx[:])
            nc.vector.tensor_tensor(rat[:], rat[:], mn[:], mybir.AluOpType.mult)
            u_t = epool.tile([N, BK], F32, tag="u_t", name="u_t", bufs=2)
            nc.vector.tensor_tensor(u_t[:], rat[:], rat[:], mybir.AluOpType.mult)
            th = epool.tile([N, BK], F32, tag="th", name="th", bufs=2)
            nc.vector.tensor_scalar_mul(th[:], u_t[:], AT_C[7])
            for ci in (6, 5, 4, 3, 2, 1):
                nc.vector.scalar_tensor_tensor(
                    th[:], th[:], AT_C[ci], u_t[:],
                    mybir.AluOpType.add, mybir.AluOpType.mult)
            nc.vector.scalar_tensor_tensor(
                th[:], th[:], AT_C[0], rat[:],
                mybir.AluOpType.add, mybir.AluOpType.mult)
            cond = epool.tile([N, BK], U8, tag="cond", name="cond", bufs=2)
            alt = epool.tile([N, BK], F32, tag="alt", name="alt", bufs=2)
            sgn = epool.tile([N, BK], F32, tag="sgn", name="sgn", bufs=2)
            nc.vector.tensor_tensor(cond[:], ax[:], ay[:], mybir.AluOpType.is_gt)
            nc.vector.tensor_scalar(alt[:], th[:], -1.0, PI / 2, mybir.AluOpType.mult, mybir.AluOpType.add)
            nc.vector.select(th[:], cond[:], alt[:], th[:])
            nc.vector.tensor_scalar(cond[:], cyf, 0.0, None, mybir.AluOpType.is_lt)
            nc.vector.tensor_scalar(alt[:], th[:], -1.0, PI, mybir.AluOpType.mult, mybir.AluOpType.add)
            nc.vector.select(th[:], cond[:], alt[:], th[:])
            nc.vector.tensor_scalar(sgn[:], cxf, 0.0, 2.0, mybir.AluOpType.is_ge, mybir.AluOpType.mult)
            nc.vector.tensor_scalar_add(sgn[:], sgn[:], -1.0)
            nc.vector.tensor_tensor(th[:], th[:], sgn[:], mybir.AluOpType.mult)

            # gaussian on [N, (b k), m]
            rho_v = rho[:].rearrange("p (x) -> p x").unsqueeze(2).broadcast_to([N, BK, M])
            th_v = th[:].unsqueeze(2).broadcast_to([N, BK, M])
            mu_r_v = mu_r.unsqueeze(1).broadcast_to([N, BK, M])
            mu_t_v = mu_t.unsqueeze(1).broadcast_to([N, BK, M])
            s_r_v = s_r.unsqueeze(1).broadcast_to([N, BK, M])
            s_t_v = s_t.unsqueeze(1).broadcast_to([N, BK, M])
            e1 = epool.tile([N, BK, M], F32, tag="e1", name="e1", bufs=2)
            nc.vector.tensor_tensor(e1[:], rho_v, mu_r_v, mybir.AluOpType.subtract)
            nc.vector.tensor_tensor(e1[:], e1[:], e1[:], mybir.AluOpType.mult)
            nc.vector.tensor_tensor(e1[:], e1[:], s_r_v, mybir.AluOpType.mult)
            e2 = epool.tile([N, BK, M], F32, tag="e2", name="e2", bufs=2)
            nc.vector.tensor_tensor(e2[:], th_v, mu_t_v, mybir.AluOpType.subtract)
            e2f = e2[:].rearrange("p a b -> p (a b)")
            nc.vector.add_range_wrap(e2f, e2f, 0.0, PI, TWO_PI)
            nc.vector.tensor_tensor(e2[:], e2[:], e2[:], mybir.AluOpType.mult)
            nc.vector.tensor_tensor(e2[:], e2[:], s_t_v, mybir.AluOpType.mult)
            nc.vector.tensor_tensor(e1[:], e1[:], e2[:], mybir.AluOpType.add)
            wg = epool.tile([N, BK, M], F32, tag="wg", name="wg", bufs=2)
            nc.scalar.activation(
                wg[:].rearrange("p a b -> p (a b)"),
                e1[:].rearrange("p a b -> p (a b)"),
                mybir.ActivationFunctionType.Exp)
            ssum = epool.tile([N, BK], F32, tag="ssum", name="ssum", bufs=2)
            nc.vector.tensor_reduce(ssum[:], wg[:], mybir.AxisListType.X, mybir.AluOpType.add)
            gws = epool.tile([N, BK], F32, tag="gws", name="gws", bufs=2)
            nc.vector.reciprocal(gws[:], ssum[:])
            nc.vector.tensor_tensor(
                gws[:], gws[:], gw_t[:].rearrange("p a b -> p (a b)"), mybir.AluOpType.mult)
            wt = epool.tile([N, BPC, K, M], BF16, tag="wt", name="wt", bufs=2)
            nc.vector.tensor_tensor(
                wt[:].rearrange("p a b c -> p (a b) c"),
                wg[:], gws[:].unsqueeze(2).broadcast_to([N, BK, M]),
                mybir.AluOpType.mult)

            # ---- phase 2: X prep (PE transpose) + F matmuls ----
            if rep == 0:
                for ct in range(C // 128):
                    eng = nc.sync if ct % 2 == 0 else nc.scalar
                    eng.dma_start(wmat[:, ct, :], wmat_v[:, ct, :])
            for b in range(BPC):
                v = st[b]
                x_bf = xpool.tile([N, C], BF16, tag="x_bf", name="x_bf")
                nc.scalar.copy(x_bf[:], v["x_f"][:])
                tp_ps = pp_fo.tile([N, C // 128, N], BF16, tag="fo", name="tp_ps")
                for ct in range(C // 128):
                    nc.tensor.transpose(tp_ps[:, ct, :], x_bf[:, ct * 128:(ct + 1) * 128], ident[:])
                xT = xpool.tile([128, C // 128, N], BF16, tag="xT", name="xT")
                nc.scalar.copy(xT[:].rearrange("p a b -> p (a b)"), tp_ps[:].rearrange("p a b -> p (a b)"))
                f_ps = pp_fo.tile([N, OUT], F32, tag="fo", name="f_ps")
                for ct in range(C // 128):
                    for h in range(2):
                        nc.tensor.matmul(
                            f_ps[:, h * 512:(h + 1) * 512],
                            xT[:, ct, :],
                            wmat[:, ct, h * 512:(h + 1) * 512],
                            start=(ct == 0), stop=(ct == C // 128 - 1))
                v["f_bf"] = xpool.tile([N, OUT], BF16, tag="f_bf", name="f_bf")
                nc.scalar.copy(v["f_bf"][:], f_ps[:])

            # ---- phase 3: Wdelta + PE collapse + final (interleaved across b) ----
            at_ps_l = {}
            for b in range(BPC):
                at_ps_l[b] = pp_a.tile([N, N * M], F32, tag="at", name="at_ps")
                for k in range(K):
                    wd = mpool.tile([N, N, M], BF16, tag="wd", name="wd", bufs=8)
                    nc.vector.tensor_tensor(
                        wd[:],
                        idm[:].rearrange("p (a b) -> p a b", b=M),
                        wt[:, b, k, :].unsqueeze(1).broadcast_to([N, N, M]),
                        mybir.AluOpType.mult)
                    wd_f = wd[:].rearrange("p a b -> p (a b)")
                    for h in range(2):
                        nc.tensor.matmul(
                            at_ps_l[b][:, h * 512:(h + 1) * 512],
                            st[b]["mask1"][:, k, :],
                            wd_f[:, h * 512:(h + 1) * 512],
                            start=(k == 0), stop=(k == K - 1))
                at_sb = tpool.tile([N, N * M], BF16, tag="at_sb", name="at_sb", bufs=3)
                nc.scalar.copy(at_sb[:], at_ps_l[b][:])

                o_ps = pp_fo.tile([N, OUT], F32, tag="fo", name="o_ps")
                at_v = at_sb[:].rearrange("p (a b) -> p a b", b=M)   # [j, n, m]
                for m in range(M):
                    nc.tensor.matmul(
                        o_ps[:, m * D:(m + 1) * D],
                        at_v[:, :, m],
                        st[b]["f_bf"][:, m * D:(m + 1) * D],
                        start=True, stop=True)
                y_sb = opool.tile([N, OUT], F32, tag="y", name="y_sb")
                nc.scalar.activation(y_sb[:], o_ps[:], mybir.ActivationFunctionType.Relu)
                nc.sync.dma_start(y_out[b], y_sb[:])

    nc.finalize()
    return nc


def _split3(v):
    """split fp32 array into three bf16 planes summing (almost) exactly to v"""
    import ml_dtypes
    hi = v.astype(ml_dtypes.bfloat16)
    r1 = v - hi.astype(np.float32)
    mid = r1.astype(ml_dtypes.bfloat16)
    lo = (r1 - mid.astype(np.float32)).astype(ml_dtypes.bfloat16)
    return hi, mid, lo


def _prep_shared(conv_w, mean_rho, mean_theta, precision_rho, precision_theta):
    import ml_dtypes
    wmat = np.ascontiguousarray(conv_w.transpose(1, 0, 2).reshape(C, OUT)).astype(ml_dtypes.bfloat16)
    gp = np.concatenate([mean_rho[0], mean_theta[0], precision_rho[0], precision_theta[0]]).astype(np.float32)
    gp = np.tile(gp[None, :], (N, 1))
    pcol = np.arange(N, dtype=np.float32)[:, None]
    return wmat, gp, pcol


def kernel(node_feats, node_centre, neighbor_idx, graph_weights,
           mean_rho, mean_theta, precision_rho, precision_theta, conv_w):
    import ml_dtypes
    node_feats = np.asarray(node_feats, dtype=np.float32)
    node_centre = np.asarray(node_centre, dtype=np.float32)
    neighbor_idx = np.asarray(neighbor_idx, dtype=np.int32)
    graph_weights = np.asarray(graph_weights, dtype=np.float32)

    if "nc" not in _CACHE:
        _CACHE["nc"] = _build_nc()
    nc = _CACHE["nc"]

    wmat, gp, pcol = _prep_shared(
        np.asarray(conv_w, dtype=np.float32),
        np.asarray(mean_rho, dtype=np.float32), np.asarray(mean_theta, dtype=np.float32),
        np.asarray(precision_rho, dtype=np.float32), np.asarray(precision_theta, dtype=np.float32))

    import ml_dtypes as _md
    idxT = np.ascontiguousarray(neighbor_idx.transpose(0, 2, 1)).astype(_md.bfloat16)
    xh, xm, xl = _split3(node_centre[..., 0])
    yh, ym, yl = _split3(node_centre[..., 1])
    c6 = np.stack([xh, xm, xl, yh, ym, yl,
                   np.zeros_like(xh), np.zeros_like(xh)], axis=-1)  # [B, N, 8] bf16

    in_maps = []
    for core in range(N_CORES):
        s = slice(core * BPC, (core + 1) * BPC)
        in_maps.append({
            "x_in": node_feats[s],
            "idx_in": neighbor_idx[s],
            "idxT_in": idxT[s],
            "gw_in": graph_weights[s],
            "c6_in": c6[s],
            "ccol_in": node_centre[s],
            "gp_in": gp,
            "wmat_in": wmat,
            "pcol_in": pcol,
        })

    res = run_bass_kernel_spmd(nc, in_maps, list(range(N_CORES)),
                               trace=bool(int(os.environ.get("KERNEL_TRACE", "0"))))
    out = np.concatenate([res.results[i]["y_out"] for i in range(N_CORES)], axis=0)
    _CACHE["last_exec_time_ns"] = res.exec_time_ns
    return out


# revision 44
# speedup vs baseline: 90.6931x; 1.0009x over previous
"""Trainium2 Bass kernel for nn_CgsNodeFeat (gaussian-mixture graph conv).

kernel(**inputs) takes the full arrays, shards batch-wise across 8
NeuronCores (4 batches each), runs one SPMD Bass program via
run_bass_kernel_spmd, and reassembles the full [32, 128, 1024] output.

Math per (batch b, node n):
  centre_j   = node_centre[b, idx[b,n,k]]                 (gather)
  rho,theta  = polar(node_centre[b,n] - centre_j)
  gauss[k,m] = exp(-.5 (rho-mu_r[m])^2 s_r[m] - .5 wrap(theta-mu_t[m])^2 s_t[m])
  w[k,m]     = graph_weights[b,n,k] * gauss[k,m] / sum_m gauss[k,m]
  out[n]     = relu( concat_m( sum_k w[k,m] F[b, idx[b,n,k], m-block] ) )
  with F[b]  = node_feats[b] @ Wmat,  Wmat[c, m*128+d] = conv_w[m, c, d]

Device mapping (per core, all tiles on 128 partitions, bf16 matmuls):
  - maskT[j,(b,k,n)] = (j == idx[n,k]): one DMA partition-broadcast of the
    host-transposed index tensor + one 4x tensor_scalar is_equal
  - mask1[n,(k,j)]: is_equal against an on-device iota table
  - centre gather on PE: matmul(maskT_bk, centre6) with centres split
    hi/mid/lo in bf16 so fp32 coordinates are reproduced exactly
  - polar/atan2 (DVE poly, deg-7 in t^2) + gaussian + normalisation:
    batched across all 4 batches as [128, 64]/[128, 512] DVE ops; theta
    wrap via the add_range_wrap custom DVE op; single ACT table
    (ln/exp/copy/relu), sqrt(s) computed as exp(0.5 ln s)
  - Wdelta_k[n',(n,m)] = Ident_m(n,n') * w[n',k,m]  (DVE 2x, bf16)
  - AT_stack[j,(n,m)] = sum_k mask1_k.T @ Wdelta_k  (PE, PSUM accum)
  - F[j,(m,d)] = X^T tiles @ Wmat tiles; X^T via PE transpose
  - out[n, m*128+d] = matmul(AT[:,(:,m)], F[:, m-block]); relu on ACT

Cost-model makespan per core: ~88 us (from 151 us for the first working
version). DVE is the bottleneck engine (~61 us busy: Wdelta expansion 38,
masks ~8, gaussian/polar ~10).
"""
import os
import sys

sys.path.insert(0, "/opt/trn_rl_repo")

import numpy as np
from contextlib import ExitStack

import concourse.bass as bass
import concourse.tile as tile
from concourse import bacc, mybir
from concourse.bass_utils import run_bass_kernel_spmd

F32 = mybir.dt.float32
U8 = mybir.dt.uint8
BF16 = mybir.dt.bfloat16
I32 = mybir.dt.int32

N_CORES = 8
B, N, K, C, M, OUT = 32, 128, 16, 1024, 8, 1024
BPC = int(os.environ.get("KERNEL_BPC", B // N_CORES))   # batches per core
D = OUT // M                # 128
TWO_PI = 2.0 * float(np.pi)
EPS = 1e-14
PI = float(np.pi)

_CACHE = {}


def _build_nc():
    nc = bacc.Bacc("TRN2", target_bir_lowering=False, debug=False, num_devices=N_CORES)

    # ---- external tensors ----
    x_in = nc.dram_tensor("x_in", [BPC, N, C], F32, kind="ExternalInput")
    idx_in = nc.dram_tensor("idx_in", [BPC, N, K], I32, kind="ExternalInput")
    idxT_in = nc.dram_tensor("idxT_in", [BPC, K, N], BF16, kind="ExternalInput")
    gw_in = nc.dram_tensor("gw_in", [BPC, N, K], F32, kind="ExternalInput")
    c6_in = nc.dram_tensor("c6_in", [BPC, N, 8], BF16, kind="ExternalInput")   # xh xm xl yh ym yl pad pad
    ccol_in = nc.dram_tensor("ccol_in", [BPC, N, 2], F32, kind="ExternalInput")
    gp_in = nc.dram_tensor("gp_in", [N, 32], F32, kind="ExternalInput")        # mu_r | mu_t | p_r | p_t (replicated)
    pcol_in = nc.dram_tensor("pcol_in", [N, 1], F32, kind="ExternalInput")     # partition index column
    wmat_in = nc.dram_tensor("wmat_in", [C, OUT], BF16, kind="ExternalInput")  # [c, m*128+d]
    y_out = nc.dram_tensor("y_out", [BPC, N, OUT], F32, kind="ExternalOutput")

    with tile.TileContext(nc) as tc, ExitStack() as ctx:
        cpool = ctx.enter_context(tc.tile_pool(name="const", bufs=1))
        wpool = ctx.enter_context(tc.tile_pool(name="wmat", bufs=1))
        xpool = ctx.enter_context(tc.tile_pool(name="x", bufs=4))
        epool = ctx.enter_context(tc.tile_pool(name="edge", bufs=4))
        mpool = ctx.enter_context(tc.tile_pool(name="mask", bufs=4))
        tpool = ctx.enter_context(tc.tile_pool(name="mt", bufs=2))
        opool = ctx.enter_context(tc.tile_pool(name="out", bufs=2))
        pp_fo = ctx.enter_context(tc.tile_pool(name="ps_fo", bufs=2, space="PSUM"))
        pp_a = ctx.enter_context(tc.tile_pool(name="ps_a", bufs=2, space="PSUM"))

        # ---- constants (generated on device) ----
        I16 = mybir.dt.int16
        ii_t = cpool.tile([N, N * K], I16)
        nc.gpsimd.iota(ii_t[:], pattern=[[0, K], [1, N]], base=0, channel_multiplier=0)
        iota2 = cpool.tile([N, N * K], BF16)
        nc.vector.tensor_copy(iota2[:], ii_t[:])
        id_i = cpool.tile([N, N], I16)
        nc.gpsimd.iota(id_i[:], pattern=[[1, N]], base=0, channel_multiplier=-1)
        ident = cpool.tile([N, N], F32)
        nc.vector.tensor_scalar(ident[:], id_i[:], 0.0, None, mybir.AluOpType.is_equal)
        di_t = cpool.tile([N, N * M], I16)
        nc.gpsimd.iota(di_t[:], pattern=[[1, N], [0, M]], base=0, channel_multiplier=-1)
        idm = cpool.tile([N, N * M], BF16)
        nc.vector.tensor_scalar(idm[:], di_t[:], 0.0, None, mybir.AluOpType.is_equal)
        gp = cpool.tile([N, 32], F32)
        nc.sync.dma_start(gp[:], gp_in[:])
        pcol = cpool.tile([N, 1], F32)
        nc.sync.dma_start(pcol[:], pcol_in[:])
        wmat = wpool.tile([128, C // 128, OUT], BF16)  # [p, ct, out]
        wmat_v = wmat_in[:].rearrange("(a p) o -> p a o", p=128)

        # gaussian scale tables: s = -0.5 / (eps + p^2), on [N, 8] slices
        mu_r = gp[:, 0:8]
        mu_t = gp[:, 8:16]
        sc = cpool.tile([N, 16], F32)   # [s_r | s_t]
        nc.vector.tensor_tensor(sc[:], gp[:, 16:32], gp[:, 16:32], mybir.AluOpType.mult)
        nc.vector.tensor_scalar_add(sc[:], sc[:], EPS)
        nc.vector.reciprocal(sc[:], sc[:])
        nc.vector.tensor_scalar_mul(sc[:], sc[:], -0.5)
        s_r = sc[:, 0:8]
        s_t = sc[:, 8:16]

        REPEAT = int(os.environ.get("KERNEL_REPEAT", "1"))
        AT_C = [9.999999228e-01, -3.333223260e-01, 1.997402841e-01, -1.404782037e-01,
                1.000220305e-01, -6.087445219e-02, 2.533168027e-02, -5.021058170e-03]
        BK = BPC * K      # batched edge width (64)
        BKM = BK * M      # 512

        def bview(t):
            # [N, BPC, K, ...] view helpers use raw slicing on 3/4-d tiles
            return t

        for rep in range(REPEAT):
            st = [dict() for _ in range(BPC)]

            # ---- phase 0: small loads first, then idxrep broadcast + masks ----
            gw_t = epool.tile([N, BPC, K], F32, tag="gw", name="gw", bufs=2)
            nc.sync.dma_start(gw_t[:], gw_in[:].rearrange("b n k -> n b k"))
            ccol = epool.tile([N, BPC, 2], F32, tag="ccol", name="ccol", bufs=2)
            for b in range(BPC):
                v = st[b]
                v["idx_t"] = epool.tile([N, K], I32, tag="idx", name="idx")
                nc.sync.dma_start(v["idx_t"][:], idx_in[b])
                v["c6"] = epool.tile([N, 8], BF16, tag="c6", name="c6")
                nc.sync.dma_start(v["c6"][:], c6_in[b])
                nc.sync.dma_start(ccol[:, b, :], ccol_in[b])

            for b in range(BPC):
                v = st[b]
                idx_t = v.pop("idx_t")
                idxf = epool.tile([N, K], BF16, tag="idxf", name="idxf")
                nc.vector.tensor_copy(idxf[:], idx_t[:])
                v["mask1"] = mpool.tile([N, K, N], BF16, tag="mask1", name="mask1")
                nc.vector.tensor_tensor(
                    v["mask1"][:],
                    iota2[:].rearrange("p (k j) -> p k j", k=K),
                    idxf[:].unsqueeze(2).broadcast_to([N, K, N]),
                    mybir.AluOpType.is_equal)

            idxrep = tpool.tile([N, BPC * K * N], BF16, tag="idxrep", name="idxrep", bufs=1)
            nc.scalar.dma_start(
                idxrep[:],
                idxT_in[:].rearrange("b k n -> (b k n)").unsqueeze(0).broadcast_to([N, BPC * K * N]))
            maskT = mpool.tile([N, BPC, K, N], BF16, tag="maskT", name="maskT", bufs=1)
            nc.vector.tensor_scalar(
                maskT[:].rearrange("p a b c -> p (a b c)"), idxrep[:], pcol[:], None,
                mybir.AluOpType.is_equal)

            for b in range(BPC):
                v = st[b]
                v["x_f"] = xpool.tile([N, C], F32, tag="x_f", name="x_f")
                nc.sync.dma_start(v["x_f"][:], x_in[b])

            # ---- phase 1: centre gather (PE) + batched polar/gaussian (DVE) ----
            cg_ps = pp_a.tile([N, BPC, K, 8], F32, tag="at", name="cg")
            for b in range(BPC):
                for k in range(K):
                    nc.tensor.matmul(cg_ps[:, b, k, :], maskT[:, b, k, :],
                                     st[b]["c6"][:], start=True, stop=True)
            cenj = epool.tile([N, BPC, K, 2], F32, tag="cenj", name="cenj", bufs=2)
            nc.vector.tensor_reduce(
                cenj[:].rearrange("p a b c -> p (a b c)"),
                cg_ps[:, :, :, 0:6].rearrange("p a k (c s) -> p a k c s", s=3),
                mybir.AxisListType.X, mybir.AluOpType.add)

            # polar: all batches at once on [N, BK]
            cx = epool.tile([N, BPC, K], F32, tag="cx", name="cx", bufs=2)
            nc.vector.tensor_tensor(
                cx[:], ccol[:, :, 0].unsqueeze(2).broadcast_to([N, BPC, K]),
                cenj[:, :, :, 0], mybir.AluOpType.subtract)
            cy = epool.tile([N, BPC, K], F32, tag="cy", name="cy", bufs=2)
            nc.vector.tensor_tensor(
                cy[:], ccol[:, :, 1].unsqueeze(2).broadcast_to([N, BPC, K]),
                cenj[:, :, :, 1], mybir.AluOpType.subtract)
            cxf = cx[:].rearrange("p a b -> p (a b)")
            cyf = cy[:].rearrange("p a b -> p (a b)")

            rho = epool.tile([N, BK], F32, tag="rho", name="rho", bufs=2)
            nc.vector.tensor_tensor(rho[:], cxf, cxf, mybir.AluOpType.mult)
            t0 = epool.tile([N, BK], F32, tag="t0", name="t0", bufs=2)
            nc.vector.tensor_tensor(t0[:], cyf, cyf, mybir.AluOpType.mult)
            nc.vector.tensor_tensor(rho[:], rho[:], t0[:], mybir.AluOpType.add)
            nc.vector.tensor_scalar_add(rho[:], rho[:], 1e-37)
            nc.scalar.activation(rho[:], rho[:], mybir.ActivationFunctionType.Ln)
            nc.scalar.activation(rho[:], rho[:], mybir.ActivationFunctionType.Exp, scale=0.5)

            ax = epool.tile([N, BK], F32, tag="ax", name="ax", bufs=2)
            nc.vector.tensor_scalar_mul(ax[:], cxf, -1.0)
            nc.vector.tensor_tensor(ax[:], ax[:], cxf, mybir.AluOpType.max)
            ay = epool.tile([N, BK], F32, tag="ay", name="ay", bufs=2)
            nc.vector.tensor_scalar_mul(ay[:], cyf, -1.0)
            nc.vector.tensor_tensor(ay[:], ay[:], cyf, mybir.AluOpType.max)
            mn = epool.tile([N, BK], F32, tag="mn", name="mn", bufs=2)
            nc.vector.tensor_tensor(mn[:], ax[:], ay[:], mybir.AluOpType.min)
            mx = epool.tile([N, BK], F32, tag="mx", name="mx", bufs=2)
            nc.vector.tensor_tensor(mx[:], ax[:], ay[:], mybir.AluOpType.max)
            nc.vector.tensor_scalar_add(mx[:], mx[:], 1e-37)
            rat = epool.tile([N, BK], F32, tag="rat", name="rat", bufs=2)
            nc.vector.reciprocal(rat[:], mx[:])
            nc.vector.tensor_tensor(rat[:], rat[:], mn[:], mybir.AluOpType.mult)
            u_t = epool.tile([N, BK], F32, tag="u_t", name="u_t", bufs=2)
            nc.vector.tensor_tensor(u_t[:], rat[:], rat[:], mybir.AluOpType.mult)
            th = epool.tile([N, BK], F32, tag="th", name="th", bufs=2)
            nc.vector.tensor_scalar_mul(th[:], u_t[:], AT_C[7])
            for ci in (6, 5, 4, 3, 2, 1):
                nc.vector.scalar_tensor_tensor(
                    th[:], th[:], AT_C[ci], u_t[:],
                    mybir.AluOpType.add, mybir.AluOpType.mult)
            nc.vector.scalar_tensor_tensor(
                th[:], th[:], AT_C[0], rat[:],
                mybir.AluOpType.add, mybir.AluOpType.mult)
            cond = epool.tile([N, BK], U8, tag="cond", name="cond", bufs=2)
            alt = epool.tile([N, BK], F32, tag="alt", name="alt", bufs=2)
            sgn = epool.tile([N, BK], F32, tag="sgn", name="sgn", bufs=2)
            nc.vector.tensor_tensor(cond[:], ax[:], ay[:], mybir.AluOpType.is_gt)
            nc.vector.tensor_scalar(alt[:], th[:], -1.0, PI / 2, mybir.AluOpType.mult, mybir.AluOpType.add)
            nc.vector.select(th[:], cond[:], alt[:], th[:])
            nc.vector.tensor_scalar(cond[:], cyf, 0.0, None, mybir.AluOpType.is_lt)
            nc.vector.tensor_scalar(alt[:], th[:], -1.0, PI, mybir.AluOpType.mult, mybir.AluOpType.add)
            nc.vector.select(th[:], cond[:], alt[:], th[:])
            nc.vector.tensor_scalar(sgn[:], cxf, 0.0, 2.0, mybir.AluOpType.is_ge, mybir.AluOpType.mult)
            nc.vector.tensor_scalar_add(sgn[:], sgn[:], -1.0)
            nc.vector.tensor_tensor(th[:], th[:], sgn[:], mybir.AluOpType.mult)

            # gaussian on [N, (b k), m]
            rho_v = rho[:].rearrange("p (x) -> p x").unsqueeze(2).broadcast_to([N, BK, M])
            th_v = th[:].unsqueeze(2).broadcast_to([N, BK, M])
            mu_r_v = mu_r.unsqueeze(1).broadcast_to([N, BK, M])
            mu_t_v = mu_t.unsqueeze(1).broadcast_to([N, BK, M])
            s_r_v = s_r.unsqueeze(1).broadcast_to([N, BK, M])
            s_t_v = s_t.unsqueeze(1).broadcast_to([N, BK, M])
            e1 = epool.tile([N, BK, M], F32, tag="e1", name="e1", bufs=2)
            nc.vector.tensor_tensor(e1[:], rho_v, mu_r_v, mybir.AluOpType.subtract)
            nc.vector.tensor_tensor(e1[:], e1[:], e1[:], mybir.AluOpType.mult)
            nc.vector.tensor_tensor(e1[:], e1[:], s_r_v, mybir.AluOpType.mult)
            e2 = epool.tile([N, BK, M], F32, tag="e2", name="e2", bufs=2)
            nc.vector.tensor_tensor(e2[:], th_v, mu_t_v, mybir.AluOpType.subtract)
            e2f = e2[:].rearrange("p a b -> p (a b)")
            nc.vector.add_range_wrap(e2f, e2f, 0.0, PI, TWO_PI)
            nc.vector.tensor_tensor(e2[:], e2[:], e2[:], mybir.AluOpType.mult)
            nc.vector.tensor_tensor(e2[:], e2[:], s_t_v, mybir.AluOpType.mult)
            nc.vector.tensor_tensor(e1[:], e1[:], e2[:], mybir.AluOpType.add)
            wg = epool.tile([N, BK, M], F32, tag="wg", name="wg", bufs=2)
            nc.scalar.activation(
                wg[:].rearrange("p a b -> p (a b)"),
                e1[:].rearrange("p a b -> p (a b)"),
                mybir.ActivationFunctionType.Exp)
            ssum = epool.tile([N, BK], F32, tag="ssum", name="ssum", bufs=2)
            nc.vector.tensor_reduce(ssum[:], wg[:], mybir.AxisListType.X, mybir.AluOpType.add)
            gws = epool.tile([N, BK], F32, tag="gws", name="gws", bufs=2)
            nc.vector.reciprocal(gws[:], ssum[:])
            nc.vector.tensor_tensor(
                gws[:], gws[:], gw_t[:].rearrange("p a b -> p (a b)"), mybir.AluOpType.mult)
            wt = epool.tile([N, BPC, K, M], BF16, tag="wt", name="wt", bufs=2)
            nc.vector.tensor_tensor(
                wt[:].rearrange("p a b c -> p (a b) c"),
                wg[:], gws[:].unsqueeze(2).broadcast_to([N, BK, M]),
                mybir.AluOpType.mult)

            # ---- phase 2: X prep (PE transpose) + F matmuls ----
            if rep == 0:
                for ct in range(C // 128):
                    eng = nc.sync if ct % 2 == 0 else nc.scalar
                    eng.dma_start(wmat[:, ct, :], wmat_v[:, ct, :])
            for b in range(BPC):
                v = st[b]
                tp_ps = pp_fo.tile([N, C // 128, N], F32, tag="fo", name="tp_ps")
                for ct in range(C // 128):
                    nc.tensor.transpose(tp_ps[:, ct, :], v["x_f"][:, ct * 128:(ct + 1) * 128], ident[:])
                xT = xpool.tile([128, C // 128, N], BF16, tag="xT", name="xT")
                nc.scalar.copy(xT[:].rearrange("p a b -> p (a b)"), tp_ps[:].rearrange("p a b -> p (a b)"))
                f_ps = pp_fo.tile([N, OUT], F32, tag="fo", name="f_ps")
                for ct in range(C // 128):
                    for h in range(2):
                        nc.tensor.matmul(
                            f_ps[:, h * 512:(h + 1) * 512],
                            xT[:, ct, :],
                            wmat[:, ct, h * 512:(h + 1) * 512],
                            start=(ct == 0), stop=(ct == C // 128 - 1))
                v["f_bf"] = xpool.tile([N, OUT], BF16, tag="f_bf", name="f_bf")
                nc.scalar.copy(v["f_bf"][:], f_ps[:])

            # ---- phase 3: Wdelta + PE collapse + final (interleaved across b) ----
            at_ps_l = {}
            for b in range(BPC):
                at_ps_l[b] = pp_a.tile([N, N * M], F32, tag="at", name="at_ps")
                for k in range(K):
                    wd = mpool.tile([N, N, M], BF16, tag="wd", name="wd", bufs=8)
                    nc.vector.tensor_tensor(
                        wd[:],
                        idm[:].rearrange("p (a b) -> p a b", b=M),
                        wt[:, b, k, :].unsqueeze(1).broadcast_to([N, N, M]),
                        mybir.AluOpType.mult)
                    wd_f = wd[:].rearrange("p a b -> p (a b)")
                    for h in range(2):
                        nc.tensor.matmul(
                            at_ps_l[b][:, h * 512:(h + 1) * 512],
                            st[b]["mask1"][:, k, :],
                            wd_f[:, h * 512:(h + 1) * 512],
                            start=(k == 0), stop=(k == K - 1))
                at_sb = tpool.tile([N, N * M], BF16, tag="at_sb", name="at_sb", bufs=3)
                nc.scalar.copy(at_sb[:], at_ps_l[b][:])

                o_ps = pp_fo.tile([N, OUT], F32, tag="fo", name="o_ps")
                at_v = at_sb[:].rearrange("p (a b) -> p a b", b=M)   # [j, n, m]
                for m in range(M):
                    nc.tensor.matmul(
                        o_ps[:, m * D:(m + 1) * D],
                        at_v[:, :, m],
                        st[b]["f_bf"][:, m * D:(m + 1) * D],
                        start=True, stop=True)
                y_sb = opool.tile([N, OUT], F32, tag="y", name="y_sb")
                nc.scalar.activation(y_sb[:], o_ps[:], mybir.ActivationFunctionType.Relu)
                nc.sync.dma_start(y_out[b], y_sb[:])

    nc.finalize()
    return nc


def _split3(v):
    """split fp32 array into three bf16 planes summing (almost) exactly to v"""
    import ml_dtypes
    hi = v.astype(ml_dtypes.bfloat16)
    r1 = v - hi.astype(np.float32)
    mid = r1.astype(ml_dtypes.bfloat16)
    lo = (r1 - mid.astype(np.float32)).astype(ml_dtypes.bfloat16)
    return hi, mid, lo


def _prep_shared(conv_w, mean_rho, mean_theta, precision_rho, precision_theta):
    import ml_dtypes
    wmat = np.ascontiguousarray(conv_w.transpose(1, 0, 2).reshape(C, OUT)).astype(ml_dtypes.bfloat16)
    gp = np.concatenate([mean_rho[0], mean_theta[0], precision_rho[0], precision_theta[0]]).astype(np.float32)
    gp = np.tile(gp[None, :], (N, 1))
    pcol = np.arange(N, dtype=np.float32)[:, None]
    return wmat, gp, pcol


def kernel(node_feats, node_centre, neighbor_idx, graph_weights,
           mean_rho, mean_theta, precision_rho, precision_theta, conv_w):
    import ml_dtypes
    node_feats = np.asarray(node_feats, dtype=np.float32)
    node_centre = np.asarray(node_centre, dtype=np.float32)
    neighbor_idx = np.asarray(neighbor_idx, dtype=np.int32)
    graph_weights = np.asarray(graph_weights, dtype=np.float32)

    if "nc" not in _CACHE:
        _CACHE["nc"] = _build_nc()
    nc = _CACHE["nc"]

    wmat, gp, pcol = _prep_shared(
        np.asarray(conv_w, dtype=np.float32),
        np.asarray(mean_rho, dtype=np.float32), np.asarray(mean_theta, dtype=np.float32),
        np.asarray(precision_rho, dtype=np.float32), np.asarray(precision_theta, dtype=np.float32))

    import ml_dtypes as _md
    idxT = np.ascontiguousarray(neighbor_idx.transpose(0, 2, 1)).astype(_md.bfloat16)
    xh, xm, xl = _split3(node_centre[..., 0])
    yh, ym, yl = _split3(node_centre[..., 1])
    c6 = np.stack([xh, xm, xl, yh, ym, yl,
                   np.zeros_like(xh), np.zeros_like(xh)], axis=-1)  # [B, N, 8] bf16

    in_maps = []
    for core in range(N_CORES):
        s = slice(core * BPC, (core + 1) * BPC)
        in_maps.append({
            "x_in": node_feats[s],
            "idx_in": neighbor_idx[s],
            "idxT_in": idxT[s],
            "gw_in": graph_weights[s],
            "c6_in": c6[s],
            "ccol_in": node_centre[s],
            "gp_in": gp,
            "wmat_in": wmat,
            "pcol_in": pcol,
        })

    res = run_bass_kernel_spmd(nc, in_maps, list(range(N_CORES)),
                               trace=bool(int(os.environ.get("KERNEL_TRACE", "0"))))
    out = np.concatenate([res.results[i]["y_out"] for i in range(N_CORES)], axis=0)
    _CACHE["last_exec_time_ns"] = res.exec_time_ns
    return out


# revision 49
# speedup vs baseline: 92.4842x; 1.0197x over previous
"""Trainium2 Bass kernel for nn_CgsNodeFeat (gaussian-mixture graph conv).

kernel(**inputs) takes the full arrays, shards batch-wise across 8
NeuronCores (4 batches each), runs one SPMD Bass program via
run_bass_kernel_spmd, and reassembles the full [32, 128, 1024] output.

Math per (batch b, node n):
  centre_j   = node_centre[b, idx[b,n,k]]                 (gather)
  rho,theta  = polar(node_centre[b,n] - centre_j)
  gauss[k,m] = exp(-.5 (rho-mu_r[m])^2 s_r[m] - .5 wrap(theta-mu_t[m])^2 s_t[m])
  w[k,m]     = graph_weights[b,n,k] * gauss[k,m] / sum_m gauss[k,m]
  out[n]     = relu( concat_m( sum_k w[k,m] F[b, idx[b,n,k], m-block] ) )
  with F[b]  = node_feats[b] @ Wmat,  Wmat[c, m*128+d] = conv_w[m, c, d]

Device mapping (per core, all tiles on 128 partitions, bf16 matmuls):
  - maskT[j,(b,k,n)] = (j == idx[n,k]): one DMA partition-broadcast of the
    host-transposed index tensor + one 4x tensor_scalar is_equal
  - mask1[n,(k,j)]: is_equal against an on-device iota table
  - centre gather on PE: matmul(maskT_bk, centre6) with centres split
    hi/mid/lo in bf16 so fp32 coordinates are reproduced exactly
  - polar/atan2 (DVE poly, deg-7 in t^2) + gaussian + normalisation:
    batched across all 4 batches as [128, 64]/[128, 512] DVE ops; theta
    wrap via the add_range_wrap custom DVE op; single ACT table
    (ln/exp/copy/relu), sqrt(s) computed as exp(0.5 ln s)
  - Wdelta_k[n',(n,m)] = Ident_m(n,n') * w[n',k,m]  (DVE 2x, bf16)
  - AT_stack[j,(n,m)] = sum_k mask1_k.T @ Wdelta_k  (PE, PSUM accum)
  - F[j,(m,d)] = X^T tiles @ Wmat tiles; X^T via PE transpose
  - out[n, m*128+d] = matmul(AT[:,(:,m)], F[:, m-block]); relu on ACT

Cost-model makespan per core: ~88 us (from 151 us for the first working
version). DVE is the bottleneck engine (~61 us busy: Wdelta expansion 38,
masks ~8, gaussian/polar ~10).
"""
import os
import sys

sys.path.insert(0, "/opt/trn_rl_repo")

import numpy as np
from contextlib import ExitStack

import concourse.bass as bass
import concourse.tile as tile
from concourse import bacc, mybir
from concourse.bass_utils import run_bass_kernel_spmd

F32 = mybir.dt.float32
U8 = mybir.dt.uint8
BF16 = mybir.dt.bfloat16
I32 = mybir.dt.int32

N_CORES = 8
B, N, K, C, M, OUT = 32, 128, 16, 1024, 8, 1024
BPC = int(os.environ.get("KERNEL_BPC", B // N_CORES))   # batches per core
D = OUT // M                # 128
TWO_PI = 2.0 * float(np.pi)
EPS = 1e-14
PI = float(np.pi)

_CACHE = {}


def _build_nc():
    nc = bacc.Bacc("TRN2", target_bir_lowering=False, debug=False, num_devices=N_CORES)

    # ---- external tensors ----
    x_in = nc.dram_tensor("x_in", [BPC, N, C], F32, kind="ExternalInput")
    idx_in = nc.dram_tensor("idx_in", [BPC, N, K], I32, kind="ExternalInput")
    idxT_in = nc.dram_tensor("idxT_in", [BPC, K, N], BF16, kind="ExternalInput")
    gw_in = nc.dram_tensor("gw_in", [BPC, N, K], F32, kind="ExternalInput")
    c6_in = nc.dram_tensor("c6_in", [BPC, N, 8], BF16, kind="ExternalInput")   # xh xm xl yh ym yl pad pad
    ccol_in = nc.dram_tensor("ccol_in", [BPC, N, 2], F32, kind="ExternalInput")
    gp_in = nc.dram_tensor("gp_in", [N, 32], F32, kind="ExternalInput")        # mu_r | mu_t | p_r | p_t (replicated)
    pcol_in = nc.dram_tensor("pcol_in", [N, 1], F32, kind="ExternalInput")     # partition index column
    wmat_in = nc.dram_tensor("wmat_in", [C, OUT], BF16, kind="ExternalInput")  # [c, m*128+d]
    y_out = nc.dram_tensor("y_out", [BPC, N, OUT], F32, kind="ExternalOutput")

    with tile.TileContext(nc) as tc, ExitStack() as ctx:
        cpool = ctx.enter_context(tc.tile_pool(name="const", bufs=1))
        wpool = ctx.enter_context(tc.tile_pool(name="wmat", bufs=1))
        xpool = ctx.enter_context(tc.tile_pool(name="x", bufs=4))
        epool = ctx.enter_context(tc.tile_pool(name="edge", bufs=4))
        mpool = ctx.enter_context(tc.tile_pool(name="mask", bufs=4))
        tpool = ctx.enter_context(tc.tile_pool(name="mt", bufs=2))
        opool = ctx.enter_context(tc.tile_pool(name="out", bufs=2))
        pp_fo = ctx.enter_context(tc.tile_pool(name="ps_fo", bufs=2, space="PSUM"))
        pp_a = ctx.enter_context(tc.tile_pool(name="ps_a", bufs=2, space="PSUM"))

        # ---- constants (generated on device) ----
        I16 = mybir.dt.int16
        ii_t = cpool.tile([N, N * K], I16)
        nc.gpsimd.iota(ii_t[:], pattern=[[0, K], [1, N]], base=0, channel_multiplier=0)
        iota2 = cpool.tile([N, N * K], BF16)
        nc.vector.tensor_copy(iota2[:], ii_t[:])
        id_i = cpool.tile([N, N], I16)
        nc.gpsimd.iota(id_i[:], pattern=[[1, N]], base=0, channel_multiplier=-1)
        ident = cpool.tile([N, N], F32)
        nc.vector.tensor_scalar(ident[:], id_i[:], 0.0, None, mybir.AluOpType.is_equal)
        di_t = cpool.tile([N, N * M], I16)
        nc.gpsimd.iota(di_t[:], pattern=[[1, N], [0, M]], base=0, channel_multiplier=-1)
        idm = cpool.tile([N, N * M], BF16)
        nc.vector.tensor_scalar(idm[:], di_t[:], 0.0, None, mybir.AluOpType.is_equal)
        gp = cpool.tile([N, 32], F32)
        nc.sync.dma_start(gp[:], gp_in[:])
        pcol = cpool.tile([N, 1], F32)
        nc.sync.dma_start(pcol[:], pcol_in[:])
        wmat = wpool.tile([128, C // 128, OUT], BF16)  # [p, ct, out]
        wmat_v = wmat_in[:].rearrange("(a p) o -> p a o", p=128)

        # gaussian scale tables: s = -0.5 / (eps + p^2), on [N, 8] slices
        mu_r = gp[:, 0:8]
        mu_t = gp[:, 8:16]
        sc = cpool.tile([N, 16], F32)   # [s_r | s_t]
        nc.vector.tensor_tensor(sc[:], gp[:, 16:32], gp[:, 16:32], mybir.AluOpType.mult)
        nc.vector.tensor_scalar_add(sc[:], sc[:], EPS)
        nc.vector.reciprocal(sc[:], sc[:])
        nc.vector.tensor_scalar_mul(sc[:], sc[:], -0.5)
        s_r = sc[:, 0:8]
        s_t = sc[:, 8:16]

        REPEAT = int(os.environ.get("KERNEL_REPEAT", "1"))
        AT_C = [9.999999228e-01, -3.333223260e-01, 1.997402841e-01, -1.404782037e-01,
                1.000220305e-01, -6.087445219e-02, 2.533168027e-02, -5.021058170e-03]
        BK = BPC * K      # batched edge width (64)
        BKM = BK * M      # 512

        def bview(t):
            # [N, BPC, K, ...] view helpers use raw slicing on 3/4-d tiles
            return t

        for rep in range(REPEAT):
            st = [dict() for _ in range(BPC)]

            # ---- phase 0: small loads first, then idxrep broadcast + masks ----
            gw_t = epool.tile([N, BPC, K], F32, tag="gw", name="gw", bufs=2)
            nc.sync.dma_start(gw_t[:], gw_in[:].rearrange("b n k -> n b k"))
            ccol = epool.tile([N, BPC, 2], F32, tag="ccol", name="ccol", bufs=2)
            for b in range(BPC):
                v = st[b]
                v["idx_t"] = epool.tile([N, K], I32, tag="idx", name="idx")
                nc.sync.dma_start(v["idx_t"][:], idx_in[b])
                v["c6"] = epool.tile([N, 8], BF16, tag="c6", name="c6")
                nc.sync.dma_start(v["c6"][:], c6_in[b])
                nc.sync.dma_start(ccol[:, b, :], ccol_in[b])

            for b in range(BPC):
                v = st[b]
                idx_t = v.pop("idx_t")
                idxf = epool.tile([N, K], BF16, tag="idxf", name="idxf")
                nc.vector.tensor_copy(idxf[:], idx_t[:])
                v["mask1"] = mpool.tile([N, K, N], BF16, tag="mask1", name="mask1")
                nc.vector.tensor_tensor(
                    v["mask1"][:],
                    iota2[:].rearrange("p (k j) -> p k j", k=K),
                    idxf[:].unsqueeze(2).broadcast_to([N, K, N]),
                    mybir.AluOpType.is_equal)

            idxrep = tpool.tile([N, BPC * K * N], BF16, tag="idxrep", name="idxrep", bufs=1)
            maskT = mpool.tile([N, BPC, K, N], BF16, tag="maskT", name="maskT", bufs=1)
            HB = BPC // 2
            for hb in range(2):
                nc.scalar.dma_start(
                    idxrep[:, hb * HB * K * N:(hb + 1) * HB * K * N],
                    idxT_in[hb * HB:(hb + 1) * HB].rearrange("b k n -> (b k n)")
                    .unsqueeze(0).broadcast_to([N, HB * K * N]))
                for b in range(hb * HB, (hb + 1) * HB):
                    nc.vector.tensor_scalar(
                        maskT[:, b, :, :].rearrange("p a b -> p (a b)"),
                        idxrep[:, b * K * N:(b + 1) * K * N], pcol[:], None,
                        mybir.AluOpType.is_equal)

            for b in range(BPC):
                v = st[b]
                v["x_f"] = xpool.tile([N, C], F32, tag="x_f", name="x_f")
                nc.sync.dma_start(v["x_f"][:], x_in[b])

            # ---- phase 1: centre gather (PE) + batched polar/gaussian (DVE) ----
            cg_ps = pp_a.tile([N, BPC, K, 8], F32, tag="at", name="cg")
            for b in range(BPC):
                for k in range(K):
                    nc.tensor.matmul(cg_ps[:, b, k, :], maskT[:, b, k, :],
                                     st[b]["c6"][:], start=True, stop=True)
            cenj = epool.tile([N, BPC, K, 2], F32, tag="cenj", name="cenj", bufs=2)
            nc.vector.tensor_reduce(
                cenj[:].rearrange("p a b c -> p (a b c)"),
                cg_ps[:, :, :, 0:6].rearrange("p a k (c s) -> p a k c s", s=3),
                mybir.AxisListType.X, mybir.AluOpType.add)

            # polar: all batches at once on [N, BK]
            cx = epool.tile([N, BPC, K], F32, tag="cx", name="cx", bufs=2)
            nc.vector.tensor_tensor(
                cx[:], ccol[:, :, 0].unsqueeze(2).broadcast_to([N, BPC, K]),
                cenj[:, :, :, 0], mybir.AluOpType.subtract)
            cy = epool.tile([N, BPC, K], F32, tag="cy", name="cy", bufs=2)
            nc.vector.tensor_tensor(
                cy[:], ccol[:, :, 1].unsqueeze(2).broadcast_to([N, BPC, K]),
                cenj[:, :, :, 1], mybir.AluOpType.subtract)
            cxf = cx[:].rearrange("p a b -> p (a b)")
            cyf = cy[:].rearrange("p a b -> p (a b)")

            rho = epool.tile([N, BK], F32, tag="rho", name="rho", bufs=2)
            nc.vector.tensor_tensor(rho[:], cxf, cxf, mybir.AluOpType.mult)
            t0 = epool.tile([N, BK], F32, tag="t0", name="t0", bufs=2)
            nc.vector.tensor_tensor(t0[:], cyf, cyf, mybir.AluOpType.mult)
            nc.vector.tensor_tensor(rho[:], rho[:], t0[:], mybir.AluOpType.add)
            nc.vector.tensor_scalar_add(rho[:], rho[:], 1e-37)
            nc.scalar.activation(rho[:], rho[:], mybir.ActivationFunctionType.Ln)
            nc.scalar.activation(rho[:], rho[:], mybir.ActivationFunctionType.Exp, scale=0.5)

            ax = epool.tile([N, BK], F32, tag="ax", name="ax", bufs=2)
            nc.vector.tensor_scalar_mul(ax[:], cxf, -1.0)
            nc.vector.tensor_tensor(ax[:], ax[:], cxf, mybir.AluOpType.max)
            ay = epool.tile([N, BK], F32, tag="ay", name="ay", bufs=2)
            nc.vector.tensor_scalar_mul(ay[:], cyf, -1.0)
            nc.vector.tensor_tensor(ay[:], ay[:], cyf, mybir.AluOpType.max)
            mn = epool.tile([N, BK], F32, tag="mn", name="mn", bufs=2)
            nc.vector.tensor_tensor(mn[:], ax[:], ay[:], mybir.AluOpType.min)
            mx = epool.tile([N, BK], F32, tag="mx", name="mx", bufs=2)
            nc.vector.tensor_tensor(mx[:], ax[:], ay[:], mybir.AluOpType.max)
            nc.vector.tensor_scalar_add(mx[:], mx[:], 1e-37)
            rat = epool.tile([N, BK], F32, tag="rat", name="rat", bufs=2)
            nc.vector.reciprocal(rat[:], mx[:])
            nc.vector.tensor_tensor(rat[:], rat[:], mn[:], mybir.AluOpType.mult)
            u_t = epool.tile([N, BK], F32, tag="u_t", name="u_t", bufs=2)
            nc.vector.tensor_tensor(u_t[:], rat[:], rat[:], mybir.AluOpType.mult)
            th = epool.tile([N, BK], F32, tag="th", name="th", bufs=2)
            nc.vector.tensor_scalar_mul(th[:], u_t[:], AT_C[7])
            for ci in (6, 5, 4, 3, 2, 1):
                nc.vector.scalar_tensor_tensor(
                    th[:], th[:], AT_C[ci], u_t[:],
                    mybir.AluOpType.add, mybir.AluOpType.mult)
            nc.vector.scalar_tensor_tensor(
                th[:], th[:], AT_C[0], rat[:],
                mybir.AluOpType.add, mybir.AluOpType.mult)
            cond = epool.tile([N, BK], U8, tag="cond", name="cond", bufs=2)
            alt = epool.tile([N, BK], F32, tag="alt", name="alt", bufs=2)
            sgn = epool.tile([N, BK], F32, tag="sgn", name="sgn", bufs=2)
            nc.vector.tensor_tensor(cond[:], ax[:], ay[:], mybir.AluOpType.is_gt)
            nc.vector.tensor_scalar(alt[:], th[:], -1.0, PI / 2, mybir.AluOpType.mult, mybir.AluOpType.add)
            nc.vector.select(th[:], cond[:], alt[:], th[:])
            nc.vector.tensor_scalar(cond[:], cyf, 0.0, None, mybir.AluOpType.is_lt)
            nc.vector.tensor_scalar(alt[:], th[:], -1.0, PI, mybir.AluOpType.mult, mybir.AluOpType.add)
            nc.vector.select(th[:], cond[:], alt[:], th[:])
            nc.vector.tensor_scalar(sgn[:], cxf, 0.0, 2.0, mybir.AluOpType.is_ge, mybir.AluOpType.mult)
            nc.vector.tensor_scalar_add(sgn[:], sgn[:], -1.0)
            nc.vector.tensor_tensor(th[:], th[:], sgn[:], mybir.AluOpType.mult)

            # gaussian on [N, (b k), m]
            rho_v = rho[:].rearrange("p (x) -> p x").unsqueeze(2).broadcast_to([N, BK, M])
            th_v = th[:].unsqueeze(2).broadcast_to([N, BK, M])
            mu_r_v = mu_r.unsqueeze(1).broadcast_to([N, BK, M])
            mu_t_v = mu_t.unsqueeze(1).broadcast_to([N, BK, M])
            s_r_v = s_r.unsqueeze(1).broadcast_to([N, BK, M])
            s_t_v = s_t.unsqueeze(1).broadcast_to([N, BK, M])
            e1 = epool.tile([N, BK, M], F32, tag="e1", name="e1", bufs=2)
            nc.vector.tensor_tensor(e1[:], rho_v, mu_r_v, mybir.AluOpType.subtract)
            nc.vector.tensor_tensor(e1[:], e1[:], e1[:], mybir.AluOpType.mult)
            nc.vector.tensor_tensor(e1[:], e1[:], s_r_v, mybir.AluOpType.mult)
            e2 = epool.tile([N, BK, M], F32, tag="e2", name="e2", bufs=2)
            nc.vector.tensor_tensor(e2[:], th_v, mu_t_v, mybir.AluOpType.subtract)
            e2f = e2[:].rearrange("p a b -> p (a b)")
            nc.vector.add_range_wrap(e2f, e2f, 0.0, PI, TWO_PI)
            nc.vector.tensor_tensor(e2[:], e2[:], e2[:], mybir.AluOpType.mult)
            nc.vector.tensor_tensor(e2[:], e2[:], s_t_v, mybir.AluOpType.mult)
            nc.vector.tensor_tensor(e1[:], e1[:], e2[:], mybir.AluOpType.add)
            wg = epool.tile([N, BK, M], F32, tag="wg", name="wg", bufs=2)
            nc.scalar.activation(
                wg[:].rearrange("p a b -> p (a b)"),
                e1[:].rearrange("p a b -> p (a b)"),
                mybir.ActivationFunctionType.Exp)
            ssum = epool.tile([N, BK], F32, tag="ssum", name="ssum", bufs=2)
            nc.vector.tensor_reduce(ssum[:], wg[:], mybir.AxisListType.X, mybir.AluOpType.add)
            gws = epool.tile([N, BK], F32, tag="gws", name="gws", bufs=2)
            nc.vector.reciprocal(gws[:], ssum[:])
            nc.vector.tensor_tensor(
                gws[:], gws[:], gw_t[:].rearrange("p a b -> p (a b)"), mybir.AluOpType.mult)
            wt = epool.tile([N, BPC, K, M], BF16, tag="wt", name="wt", bufs=2)
            nc.vector.tensor_tensor(
                wt[:].rearrange("p a b c -> p (a b) c"),
                wg[:], gws[:].unsqueeze(2).broadcast_to([N, BK, M]),
                mybir.AluOpType.mult)

            # ---- phase 2: X prep (PE transpose) + F matmuls ----
            if rep == 0:
                for ct in range(C // 128):
                    eng = nc.sync if ct % 2 == 0 else nc.scalar
                    eng.dma_start(wmat[:, ct, :], wmat_v[:, ct, :])
            for b in range(BPC):
                v = st[b]
                tp_ps = pp_fo.tile([N, C // 128, N], F32, tag="fo", name="tp_ps")
                for ct in range(C // 128):
                    nc.tensor.transpose(tp_ps[:, ct, :], v["x_f"][:, ct * 128:(ct + 1) * 128], ident[:])
                xT = xpool.tile([128, C // 128, N], BF16, tag="xT", name="xT")
                nc.scalar.copy(xT[:].rearrange("p a b -> p (a b)"), tp_ps[:].rearrange("p a b -> p (a b)"))
                f_ps = pp_fo.tile([N, OUT], F32, tag="fo", name="f_ps")
                for ct in range(C // 128):
                    for h in range(2):
                        nc.tensor.matmul(
                            f_ps[:, h * 512:(h + 1) * 512],
                            xT[:, ct, :],
                            wmat[:, ct, h * 512:(h + 1) * 512],
                            start=(ct == 0), stop=(ct == C // 128 - 1))
                v["f_bf"] = xpool.tile([N, OUT], BF16, tag="f_bf", name="f_bf")
                nc.scalar.copy(v["f_bf"][:], f_ps[:])

            # ---- phase 3: Wdelta + PE collapse + final (interleaved across b) ----
            at_ps_l = {}
            for b in range(BPC):
                at_ps_l[b] = pp_a.tile([N, N * M], F32, tag="at", name="at_ps")
                for k in range(K):
                    wd = mpool.tile([N, N, M], BF16, tag="wd", name="wd", bufs=8)
                    nc.vector.tensor_tensor(
                        wd[:],
                        idm[:].rearrange("p (a b) -> p a b", b=M),
                        wt[:, b, k, :].unsqueeze(1).broadcast_to([N, N, M]),
                        mybir.AluOpType.mult)
                    wd_f = wd[:].rearrange("p a b -> p (a b)")
                    for h in range(2):
                        nc.tensor.matmul(
                            at_ps_l[b][:, h * 512:(h + 1) * 512],
                            st[b]["mask1"][:, k, :],
                            wd_f[:, h * 512:(h + 1) * 512],
                            start=(k == 0), stop=(k == K - 1))
                at_sb = tpool.tile([N, N * M], BF16, tag="at_sb", name="at_sb", bufs=3)
                nc.scalar.copy(at_sb[:], at_ps_l[b][:])

                o_ps = pp_fo.tile([N, OUT], F32, tag="fo", name="o_ps")
                at_v = at_sb[:].rearrange("p (a b) -> p a b", b=M)   # [j, n, m]
                for m in range(M):
                    nc.tensor.matmul(
                        o_ps[:, m * D:(m + 1) * D],
                        at_v[:, :, m],
                        st[b]["f_bf"][:, m * D:(m + 1) * D],
                        start=True, stop=True)
                y_sb = opool.tile([N, OUT], F32, tag="y", name="y_sb")
                nc.scalar.activation(y_sb[:], o_ps[:], mybir.ActivationFunctionType.Relu)
                nc.sync.dma_start(y_out[b], y_sb[:])

    nc.finalize()
    return nc


def _split3(v):
    """split fp32 array into three bf16 planes summing (almost) exactly to v"""
    import ml_dtypes
    hi = v.astype(ml_dtypes.bfloat16)
    r1 = v - hi.astype(np.float32)
    mid = r1.astype(ml_dtypes.bfloat16)
    lo = (r1 - mid.astype(np.float32)).astype(ml_dtypes.bfloat16)
    return hi, mid, lo


def _prep_shared(conv_w, mean_rho, mean_theta, precision_rho, precision_theta):
    import ml_dtypes
    wmat = np.ascontiguousarray(conv_w.transpose(1, 0, 2).reshape(C, OUT)).astype(ml_dtypes.bfloat16)
    gp = np.concatenate([mean_rho[0], mean_theta[0], precision_rho[0], precision_theta[0]]).astype(np.float32)
    gp = np.tile(gp[None, :], (N, 1))
    pcol = np.arange(N, dtype=np.float32)[:, None]
    return wmat, gp, pcol


def kernel(node_feats, node_centre, neighbor_idx, graph_weights,
           mean_rho, mean_theta, precision_rho, precision_theta, conv_w):
    import ml_dtypes
    node_feats = np.asarray(node_feats, dtype=np.float32)
    node_centre = np.asarray(node_centre, dtype=np.float32)
    neighbor_idx = np.asarray(neighbor_idx, dtype=np.int32)
    graph_weights = np.asarray(graph_weights, dtype=np.float32)

    if "nc" not in _CACHE:
        _CACHE["nc"] = _build_nc()
    nc = _CACHE["nc"]

    wmat, gp, pcol = _prep_shared(
        np.asarray(conv_w, dtype=np.float32),
        np.asarray(mean_rho, dtype=np.float32), np.asarray(mean_theta, dtype=np.float32),
        np.asarray(precision_rho, dtype=np.float32), np.asarray(precision_theta, dtype=np.float32))

    import ml_dtypes as _md
    idxT = np.ascontiguousarray(neighbor_idx.transpose(0, 2, 1)).astype(_md.bfloat16)
    xh, xm, xl = _split3(node_centre[..., 0])
    yh, ym, yl = _split3(node_centre[..., 1])
    c6 = np.stack([xh, xm, xl, yh, ym, yl,
                   np.zeros_like(xh), np.zeros_like(xh)], axis=-1)  # [B, N, 8] bf16

    in_maps = []
    for core in range(N_CORES):
        s = slice(core * BPC, (core + 1) * BPC)
        in_maps.append({
            "x_in": node_feats[s],
            "idx_in": neighbor_idx[s],
            "idxT_in": idxT[s],
            "gw_in": graph_weights[s],
            "c6_in": c6[s],
            "ccol_in": node_centre[s],
            "gp_in": gp,
            "wmat_in": wmat,
            "pcol_in": pcol,
        })

    res = run_bass_kernel_spmd(nc, in_maps, list(range(N_CORES)),
                               trace=bool(int(os.environ.get("KERNEL_TRACE", "0"))))
    out = np.concatenate([res.results[i]["y_out"] for i in range(N_CORES)], axis=0)
    _CACHE["last_exec_time_ns"] = res.exec_time_ns
    return out


# revision 51
# speedup vs baseline: 92.9227x; 1.0047x over previous
"""Trainium2 Bass kernel for nn_CgsNodeFeat (gaussian-mixture graph conv).

kernel(**inputs) takes the full arrays, shards batch-wise across 8
NeuronCores (4 batches each), runs one SPMD Bass program via
run_bass_kernel_spmd, and reassembles the full [32, 128, 1024] output.

Math per (batch b, node n):
  centre_j   = node_centre[b, idx[b,n,k]]                 (gather)
  rho,theta  = polar(node_centre[b,n] - centre_j)
  gauss[k,m] = exp(-.5 (rho-mu_r[m])^2 s_r[m] - .5 wrap(theta-mu_t[m])^2 s_t[m])
  w[k,m]     = graph_weights[b,n,k] * gauss[k,m] / sum_m gauss[k,m]
  out[n]     = relu( concat_m( sum_k w[k,m] F[b, idx[b,n,k], m-block] ) )
  with F[b]  = node_feats[b] @ Wmat,  Wmat[c, m*128+d] = conv_w[m, c, d]

Device mapping (per core, all tiles on 128 partitions, bf16 matmuls):
  - maskT[j,(b,k,n)] = (j == idx[n,k]): one DMA partition-broadcast of the
    host-transposed index tensor + one 4x tensor_scalar is_equal
  - mask1[n,(k,j)]: is_equal against an on-device iota table
  - centre gather on PE: matmul(maskT_bk, centre6) with centres split
    hi/mid/lo in bf16 so fp32 coordinates are reproduced exactly
  - polar/atan2 (DVE poly, deg-7 in t^2) + gaussian + normalisation:
    batched across all 4 batches as [128, 64]/[128, 512] DVE ops; theta
    wrap via the add_range_wrap custom DVE op; single ACT table
    (ln/exp/copy/relu), sqrt(s) computed as exp(0.5 ln s)
  - Wdelta_k[n',(n,m)] = Ident_m(n,n') * w[n',k,m]  (DVE 2x, bf16)
  - AT_stack[j,(n,m)] = sum_k mask1_k.T @ Wdelta_k  (PE, PSUM accum)
  - F[j,(m,d)] = X^T tiles @ Wmat tiles; X^T via PE transpose
  - out[n, m*128+d] = matmul(AT[:,(:,m)], F[:, m-block]); relu on ACT

Cost-model makespan per core: ~88 us (from 151 us for the first working
version). DVE is the bottleneck engine (~61 us busy: Wdelta expansion 38,
masks ~8, gaussian/polar ~10).
"""
import os
import sys

sys.path.insert(0, "/opt/trn_rl_repo")

import numpy as np
from contextlib import ExitStack

import concourse.bass as bass
import concourse.tile as tile
from concourse import bacc, mybir
from concourse.bass_utils import run_bass_kernel_spmd

F32 = mybir.dt.float32
U8 = mybir.dt.uint8
BF16 = mybir.dt.bfloat16
I32 = mybir.dt.int32

N_CORES = 8
B, N, K, C, M, OUT = 32, 128, 16, 1024, 8, 1024
BPC = int(os.environ.get("KERNEL_BPC", B // N_CORES))   # batches per core
D = OUT // M                # 128
TWO_PI = 2.0 * float(np.pi)
EPS = 1e-14
PI = float(np.pi)

_CACHE = {}


def _build_nc():
    nc = bacc.Bacc("TRN2", target_bir_lowering=False, debug=False, num_devices=N_CORES)

    # ---- external tensors ----
    x_in = nc.dram_tensor("x_in", [BPC, N, C], F32, kind="ExternalInput")
    idx_in = nc.dram_tensor("idx_in", [BPC, N, K], I32, kind="ExternalInput")
    idxT_in = nc.dram_tensor("idxT_in", [BPC, K, N], BF16, kind="ExternalInput")
    gw_in = nc.dram_tensor("gw_in", [BPC, N, K], F32, kind="ExternalInput")
    c6_in = nc.dram_tensor("c6_in", [BPC, N, 8], BF16, kind="ExternalInput")   # xh xm xl yh ym yl pad pad
    ccol_in = nc.dram_tensor("ccol_in", [BPC, N, 2], F32, kind="ExternalInput")
    gp_in = nc.dram_tensor("gp_in", [N, 32], F32, kind="ExternalInput")        # mu_r | mu_t | p_r | p_t (replicated)
    pcol_in = nc.dram_tensor("pcol_in", [N, 1], F32, kind="ExternalInput")     # partition index column
    wmat_in = nc.dram_tensor("wmat_in", [C, OUT], BF16, kind="ExternalInput")  # [c, m*128+d]
    y_out = nc.dram_tensor("y_out", [BPC, N, OUT], F32, kind="ExternalOutput")

    with tile.TileContext(nc) as tc, ExitStack() as ctx:
        cpool = ctx.enter_context(tc.tile_pool(name="const", bufs=1))
        wpool = ctx.enter_context(tc.tile_pool(name="wmat", bufs=1))
        xpool = ctx.enter_context(tc.tile_pool(name="x", bufs=4))
        epool = ctx.enter_context(tc.tile_pool(name="edge", bufs=4))
        mpool = ctx.enter_context(tc.tile_pool(name="mask", bufs=4))
        tpool = ctx.enter_context(tc.tile_pool(name="mt", bufs=2))
        opool = ctx.enter_context(tc.tile_pool(name="out", bufs=2))
        pp_fo = ctx.enter_context(tc.tile_pool(name="ps_fo", bufs=2, space="PSUM"))
        pp_a = ctx.enter_context(tc.tile_pool(name="ps_a", bufs=2, space="PSUM"))

        # ---- constants (generated on device) ----
        I16 = mybir.dt.int16
        ii_t = cpool.tile([N, N * K], I16)
        nc.gpsimd.iota(ii_t[:], pattern=[[0, K], [1, N]], base=0, channel_multiplier=0)
        iota2 = cpool.tile([N, N * K], BF16)
        nc.vector.tensor_copy(iota2[:], ii_t[:])
        id_i = cpool.tile([N, N], I16)
        nc.gpsimd.iota(id_i[:], pattern=[[1, N]], base=0, channel_multiplier=-1)
        ident = cpool.tile([N, N], F32)
        nc.vector.tensor_scalar(ident[:], id_i[:], 0.0, None, mybir.AluOpType.is_equal)
        di_t = cpool.tile([N, N * M], I16)
        nc.gpsimd.iota(di_t[:], pattern=[[1, N], [0, M]], base=0, channel_multiplier=-1)
        idm = cpool.tile([N, N * M], BF16)
        nc.vector.tensor_scalar(idm[:], di_t[:], 0.0, None, mybir.AluOpType.is_equal)
        gp = cpool.tile([N, 32], F32)
        nc.sync.dma_start(gp[:], gp_in[:])
        pcol = cpool.tile([N, 1], F32)
        nc.sync.dma_start(pcol[:], pcol_in[:])
        wmat = wpool.tile([128, C // 128, OUT], BF16)  # [p, ct, out]
        wmat_v = wmat_in[:].rearrange("(a p) o -> p a o", p=128)

        # gaussian scale tables: s = -0.5 / (eps + p^2), on [N, 8] slices
        mu_r = gp[:, 0:8]
        mu_t = gp[:, 8:16]
        sc = cpool.tile([N, 16], F32)   # [s_r | s_t]
        nc.vector.tensor_tensor(sc[:], gp[:, 16:32], gp[:, 16:32], mybir.AluOpType.mult)
        nc.vector.tensor_scalar_add(sc[:], sc[:], EPS)
        nc.vector.reciprocal(sc[:], sc[:])
        nc.vector.tensor_scalar_mul(sc[:], sc[:], -0.5)
        s_r = sc[:, 0:8]
        s_t = sc[:, 8:16]

        REPEAT = int(os.environ.get("KERNEL_REPEAT", "1"))
        AT_C = [9.999999228e-01, -3.333223260e-01, 1.997402841e-01, -1.404782037e-01,
                1.000220305e-01, -6.087445219e-02, 2.533168027e-02, -5.021058170e-03]
        BK = BPC * K      # batched edge width (64)
        BKM = BK * M      # 512

        def bview(t):
            # [N, BPC, K, ...] view helpers use raw slicing on 3/4-d tiles
            return t

        for rep in range(REPEAT):
            st = [dict() for _ in range(BPC)]

            # ---- phase 0: small loads first, then idxrep broadcast + masks ----
            gw_t = epool.tile([N, BPC, K], F32, tag="gw", name="gw", bufs=2)
            nc.sync.dma_start(gw_t[:], gw_in[:].rearrange("b n k -> n b k"))
            ccol = epool.tile([N, BPC, 2], F32, tag="ccol", name="ccol", bufs=2)
            for b in range(BPC):
                v = st[b]
                v["idx_t"] = epool.tile([N, K], I32, tag="idx", name="idx")
                nc.sync.dma_start(v["idx_t"][:], idx_in[b])
                v["c6"] = epool.tile([N, 8], BF16, tag="c6", name="c6")
                nc.sync.dma_start(v["c6"][:], c6_in[b])
                nc.sync.dma_start(ccol[:, b, :], ccol_in[b])

            for b in range(BPC):
                v = st[b]
                idx_t = v.pop("idx_t")
                idxf = epool.tile([N, K], BF16, tag="idxf", name="idxf")
                nc.vector.tensor_copy(idxf[:], idx_t[:])
                v["mask1"] = mpool.tile([N, K, N], BF16, tag="mask1", name="mask1")
                nc.vector.tensor_tensor(
                    v["mask1"][:],
                    iota2[:].rearrange("p (k j) -> p k j", k=K),
                    idxf[:].unsqueeze(2).broadcast_to([N, K, N]),
                    mybir.AluOpType.is_equal)

            idxrep = tpool.tile([N, BPC * K * N], BF16, tag="idxrep", name="idxrep", bufs=1)
            maskT = mpool.tile([N, BPC, K, N], BF16, tag="maskT", name="maskT", bufs=1)
            HB = BPC // 2
            for hb in range(2):
                nc.scalar.dma_start(
                    idxrep[:, hb * HB * K * N:(hb + 1) * HB * K * N],
                    idxT_in[hb * HB:(hb + 1) * HB].rearrange("b k n -> (b k n)")
                    .unsqueeze(0).broadcast_to([N, HB * K * N]))
                for b in range(hb * HB, (hb + 1) * HB):
                    nc.vector.tensor_scalar(
                        maskT[:, b, :, :].rearrange("p a b -> p (a b)"),
                        idxrep[:, b * K * N:(b + 1) * K * N], pcol[:], None,
                        mybir.AluOpType.is_equal)

            for b in range(BPC):
                v = st[b]
                v["x_f"] = xpool.tile([N, C], F32, tag="x_f", name="x_f")
                nc.sync.dma_start(v["x_f"][:], x_in[b])

            # ---- phase 1: centre gather (PE) + batched polar/gaussian (DVE) ----
            cg_ps = pp_a.tile([N, BPC, K, 8], F32, tag="at", name="cg")
            for b in range(BPC):
                for k in range(K):
                    nc.tensor.matmul(cg_ps[:, b, k, :], maskT[:, b, k, :],
                                     st[b]["c6"][:], start=True, stop=True)
            cenj = epool.tile([N, BPC, K, 2], F32, tag="cenj", name="cenj", bufs=2)
            nc.vector.tensor_reduce(
                cenj[:].rearrange("p a b c -> p (a b c)"),
                cg_ps[:, :, :, 0:6].rearrange("p a k (c s) -> p a k c s", s=3),
                mybir.AxisListType.X, mybir.AluOpType.add)

            # polar: all batches at once on [N, BK]
            cx = epool.tile([N, BPC, K], F32, tag="cx", name="cx", bufs=2)
            nc.vector.tensor_tensor(
                cx[:], ccol[:, :, 0].unsqueeze(2).broadcast_to([N, BPC, K]),
                cenj[:, :, :, 0], mybir.AluOpType.subtract)
            cy = epool.tile([N, BPC, K], F32, tag="cy", name="cy", bufs=2)
            nc.vector.tensor_tensor(
                cy[:], ccol[:, :, 1].unsqueeze(2).broadcast_to([N, BPC, K]),
                cenj[:, :, :, 1], mybir.AluOpType.subtract)
            cxf = cx[:].rearrange("p a b -> p (a b)")
            cyf = cy[:].rearrange("p a b -> p (a b)")

            rho = epool.tile([N, BK], F32, tag="rho", name="rho", bufs=2)
            nc.vector.tensor_tensor(rho[:], cxf, cxf, mybir.AluOpType.mult)
            t0 = epool.tile([N, BK], F32, tag="t0", name="t0", bufs=2)
            nc.vector.tensor_tensor(t0[:], cyf, cyf, mybir.AluOpType.mult)
            nc.vector.tensor_tensor(rho[:], rho[:], t0[:], mybir.AluOpType.add)
            nc.vector.tensor_scalar_add(rho[:], rho[:], 1e-37)
            nc.scalar.activation(rho[:], rho[:], mybir.ActivationFunctionType.Ln)
            nc.scalar.activation(rho[:], rho[:], mybir.ActivationFunctionType.Exp, scale=0.5)

            ax = epool.tile([N, BK], F32, tag="ax", name="ax", bufs=2)
            nc.vector.tensor_scalar_mul(ax[:], cxf, -1.0)
            nc.vector.tensor_tensor(ax[:], ax[:], cxf, mybir.AluOpType.max)
            ay = epool.tile([N, BK], F32, tag="ay", name="ay", bufs=2)
            nc.vector.tensor_scalar_mul(ay[:], cyf, -1.0)
            nc.vector.tensor_tensor(ay[:], ay[:], cyf, mybir.AluOpType.max)
            mn = epool.tile([N, BK], F32, tag="mn", name="mn", bufs=2)
            nc.vector.tensor_tensor(mn[:], ax[:], ay[:], mybir.AluOpType.min)
            mx = epool.tile([N, BK], F32, tag="mx", name="mx", bufs=2)
            nc.vector.tensor_tensor(mx[:], ax[:], ay[:], mybir.AluOpType.max)
            nc.vector.tensor_scalar_add(mx[:], mx[:], 1e-37)
            rat = epool.tile([N, BK], F32, tag="rat", name="rat", bufs=2)
            nc.vector.reciprocal(rat[:], mx[:])
            nc.vector.tensor_tensor(rat[:], rat[:], mn[:], mybir.AluOpType.mult)
            u_t = epool.tile([N, BK], F32, tag="u_t", name="u_t", bufs=2)
            nc.vector.tensor_tensor(u_t[:], rat[:], rat[:], mybir.AluOpType.mult)
            th = epool.tile([N, BK], F32, tag="th", name="th", bufs=2)
            nc.vector.tensor_scalar_mul(th[:], u_t[:], AT_C[7])
            for ci in (6, 5, 4, 3, 2, 1):
                nc.vector.scalar_tensor_tensor(
                    th[:], th[:], AT_C[ci], u_t[:],
                    mybir.AluOpType.add, mybir.AluOpType.mult)
            nc.vector.scalar_tensor_tensor(
                th[:], th[:], AT_C[0], rat[:],
                mybir.AluOpType.add, mybir.AluOpType.mult)
            cond = epool.tile([N, BK], U8, tag="cond", name="cond", bufs=2)
            alt = epool.tile([N, BK], F32, tag="alt", name="alt", bufs=2)
            sgn = epool.tile([N, BK], F32, tag="sgn", name="sgn", bufs=2)
            nc.vector.tensor_tensor(cond[:], ax[:], ay[:], mybir.AluOpType.is_gt)
            nc.vector.tensor_scalar(alt[:], th[:], -1.0, PI / 2, mybir.AluOpType.mult, mybir.AluOpType.add)
            nc.vector.select(th[:], cond[:], alt[:], th[:])
            nc.vector.tensor_scalar(cond[:], cyf, 0.0, None, mybir.AluOpType.is_lt)
            nc.vector.tensor_scalar(alt[:], th[:], -1.0, PI, mybir.AluOpType.mult, mybir.AluOpType.add)
            nc.vector.select(th[:], cond[:], alt[:], th[:])
            nc.vector.tensor_scalar(sgn[:], cxf, 0.0, 2.0, mybir.AluOpType.is_ge, mybir.AluOpType.mult)
            nc.vector.tensor_scalar_add(sgn[:], sgn[:], -1.0)
            nc.vector.tensor_tensor(th[:], th[:], sgn[:], mybir.AluOpType.mult)

            # gaussian on [N, (b k), m]
            rho_v = rho[:].rearrange("p (x) -> p x").unsqueeze(2).broadcast_to([N, BK, M])
            th_v = th[:].unsqueeze(2).broadcast_to([N, BK, M])
            mu_r_v = mu_r.unsqueeze(1).broadcast_to([N, BK, M])
            mu_t_v = mu_t.unsqueeze(1).broadcast_to([N, BK, M])
            s_r_v = s_r.unsqueeze(1).broadcast_to([N, BK, M])
            s_t_v = s_t.unsqueeze(1).broadcast_to([N, BK, M])
            e1 = epool.tile([N, BK, M], F32, tag="e1", name="e1", bufs=2)
            nc.vector.tensor_tensor(e1[:], rho_v, mu_r_v, mybir.AluOpType.subtract)
            nc.vector.tensor_tensor(e1[:], e1[:], e1[:], mybir.AluOpType.mult)
            nc.vector.tensor_tensor(e1[:], e1[:], s_r_v, mybir.AluOpType.mult)
            e2 = epool.tile([N, BK, M], F32, tag="e2", name="e2", bufs=2)
            nc.vector.tensor_tensor(e2[:], th_v, mu_t_v, mybir.AluOpType.subtract)
            e2f = e2[:].rearrange("p a b -> p (a b)")
            nc.vector.add_range_wrap(e2f, e2f, 0.0, PI, TWO_PI)
            nc.vector.tensor_tensor(e2[:], e2[:], e2[:], mybir.AluOpType.mult)
            nc.vector.tensor_tensor(e2[:], e2[:], s_t_v, mybir.AluOpType.mult)
            nc.vector.tensor_tensor(e1[:], e1[:], e2[:], mybir.AluOpType.add)
            wg = epool.tile([N, BK, M], F32, tag="wg", name="wg", bufs=2)
            nc.scalar.activation(
                wg[:].rearrange("p a b -> p (a b)"),
                e1[:].rearrange("p a b -> p (a b)"),
                mybir.ActivationFunctionType.Exp)
            ssum = epool.tile([N, BK], F32, tag="ssum", name="ssum", bufs=2)
            nc.vector.tensor_reduce(ssum[:], wg[:], mybir.AxisListType.X, mybir.AluOpType.add)
            gws = epool.tile([N, BK], F32, tag="gws", name="gws", bufs=2)
            nc.vector.reciprocal(gws[:], ssum[:])
            nc.vector.tensor_tensor(
                gws[:], gws[:], gw_t[:].rearrange("p a b -> p (a b)"), mybir.AluOpType.mult)
            wt = epool.tile([N, BPC, K, M], BF16, tag="wt", name="wt", bufs=2)
            nc.vector.tensor_tensor(
                wt[:].rearrange("p a b c -> p (a b) c"),
                wg[:], gws[:].unsqueeze(2).broadcast_to([N, BK, M]),
                mybir.AluOpType.mult)

            # ---- phase 2: X prep (PE transpose) + F matmuls ----
            if rep == 0:
                for ct in range(C // 128):
                    eng = nc.sync if ct % 2 == 0 else nc.scalar
                    eng.dma_start(wmat[:, ct, :], wmat_v[:, ct, :])
            for b in range(BPC):
                v = st[b]
                tp_ps = pp_fo.tile([N, C // 128, N], F32, tag="fo", name="tp_ps")
                for ct in range(C // 128):
                    nc.tensor.transpose(tp_ps[:, ct, :], v["x_f"][:, ct * 128:(ct + 1) * 128], ident[:])
                xT = xpool.tile([128, C // 128, N], BF16, tag="xT", name="xT")
                nc.scalar.copy(xT[:].rearrange("p a b -> p (a b)"), tp_ps[:].rearrange("p a b -> p (a b)"))
                f_ps = pp_fo.tile([N, OUT], F32, tag="fo", name="f_ps")
                for ct in range(C // 128):
                    for h in range(2):
                        nc.tensor.matmul(
                            f_ps[:, h * 512:(h + 1) * 512],
                            xT[:, ct, :],
                            wmat[:, ct, h * 512:(h + 1) * 512],
                            start=(ct == 0), stop=(ct == C // 128 - 1))
                v["f_bf"] = xpool.tile([N, OUT], BF16, tag="f_bf", name="f_bf")
                nc.scalar.copy(v["f_bf"][:], f_ps[:])

            # ---- phase 3: Wdelta + PE collapse + final (interleaved across b) ----
            at_ps_l = {}
            for b in range(BPC):
                at_ps_l[b] = pp_a.tile([N, N * M], F32, tag="at", name="at_ps")
                for k in range(K):
                    wd = mpool.tile([N, N, M], BF16, tag="wd", name="wd", bufs=8)
                    nc.vector.tensor_tensor(
                        wd[:],
                        idm[:].rearrange("p (a b) -> p a b", b=M),
                        wt[:, b, k, :].unsqueeze(1).broadcast_to([N, N, M]),
                        mybir.AluOpType.mult)
                    wd_f = wd[:].rearrange("p a b -> p (a b)")
                    for h in range(2):
                        nc.tensor.matmul(
                            at_ps_l[b][:, h * 512:(h + 1) * 512],
                            st[b]["mask1"][:, k, :],
                            wd_f[:, h * 512:(h + 1) * 512],
                            start=(k == 0), stop=(k == K - 1))
                at_sb = tpool.tile([N, N * M], BF16, tag="at_sb", name="at_sb", bufs=3)
                nc.scalar.copy(at_sb[:], at_ps_l[b][:])

                o_ps = pp_fo.tile([N, OUT], F32, tag="fo", name="o_ps")
                at_v = at_sb[:].rearrange("p (a b) -> p a b", b=M)   # [j, n, m]
                for m in range(M):
                    nc.tensor.matmul(
                        o_ps[:, m * D:(m + 1) * D],
                        at_v[:, :, m],
                        st[b]["f_bf"][:, m * D:(m + 1) * D],
                        start=True, stop=True)
                y_sb = opool.tile([N, OUT], F32, tag="y", name="y_sb")
                for yh in range(2):
                    ysl = slice(yh * OUT // 2, (yh + 1) * OUT // 2)
                    nc.scalar.activation(y_sb[:, ysl], o_ps[:, ysl], mybir.ActivationFunctionType.Relu)
                    nc.sync.dma_start(y_out[b][:, ysl], y_sb[:, ysl])

    nc.finalize()
    return nc


def _split3(v):
    """split fp32 array into three bf16 planes summing (almost) exactly to v"""
    import ml_dtypes
    hi = v.astype(ml_dtypes.bfloat16)
    r1 = v - hi.astype(np.float32)
    mid = r1.astype(ml_dtypes.bfloat16)
    lo = (r1 - mid.astype(np.float32)).astype(ml_dtypes.bfloat16)
    return hi, mid, lo


def _prep_shared(conv_w, mean_rho, mean_theta, precision_rho, precision_theta):
    import ml_dtypes
    wmat = np.ascontiguousarray(conv_w.transpose(1, 0, 2).reshape(C, OUT)).astype(ml_dtypes.bfloat16)
    gp = np.concatenate([mean_rho[0], mean_theta[0], precision_rho[0], precision_theta[0]]).astype(np.float32)
    gp = np.tile(gp[None, :], (N, 1))
    pcol = np.arange(N, dtype=np.float32)[:, None]
    return wmat, gp, pcol


def kernel(node_feats, node_centre, neighbor_idx, graph_weights,
           mean_rho, mean_theta, precision_rho, precision_theta, conv_w):
    import ml_dtypes
    node_feats = np.asarray(node_feats, dtype=np.float32)
    node_centre = np.asarray(node_centre, dtype=np.float32)
    neighbor_idx = np.asarray(neighbor_idx, dtype=np.int32)
    graph_weights = np.asarray(graph_weights, dtype=np.float32)

    if "nc" not in _CACHE:
        _CACHE["nc"] = _build_nc()
    nc = _CACHE["nc"]

    wmat, gp, pcol = _prep_shared(
        np.asarray(conv_w, dtype=np.float32),
        np.asarray(mean_rho, dtype=np.float32), np.asarray(mean_theta, dtype=np.float32),
        np.asarray(precision_rho, dtype=np.float32), np.asarray(precision_theta, dtype=np.float32))

    import ml_dtypes as _md
    idxT = np.ascontiguousarray(neighbor_idx.transpose(0, 2, 1)).astype(_md.bfloat16)
    xh, xm, xl = _split3(node_centre[..., 0])
    yh, ym, yl = _split3(node_centre[..., 1])
    c6 = np.stack([xh, xm, xl, yh, ym, yl,
                   np.zeros_like(xh), np.zeros_like(xh)], axis=-1)  # [B, N, 8] bf16

    in_maps = []
    for core in range(N_CORES):
        s = slice(core * BPC, (core + 1) * BPC)
        in_maps.append({
            "x_in": node_feats[s],
            "idx_in": neighbor_idx[s],
            "idxT_in": idxT[s],
            "gw_in": graph_weights[s],
            "c6_in": c6[s],
            "ccol_in": node_centre[s],
            "gp_in": gp,
            "wmat_in": wmat,
            "pcol_in": pcol,
        })

    res = run_bass_kernel_spmd(nc, in_maps, list(range(N_CORES)),
                               trace=bool(int(os.environ.get("KERNEL_TRACE", "0"))))
    out = np.concatenate([res.results[i]["y_out"] for i in range(N_CORES)], axis=0)
    _CACHE["last_exec_time_ns"] = res.exec_time_ns
    return out
